# revision 7
# baseline (speedup 1.0000x reference)
"""Trainium2 Bass kernel for nn_MultiHeadAttention_9491877724818.

Math (per batch b, head h), reformulated from the reference:
    q = Wq_h @ x_b + bq          (128, T)
    k = Wk_h @ x_b + bk          (128, T)
    eT[s,t] = (k.T @ q)[s,t]     == energy[t,s]; softmax over s (partition dim)
    expET = exp(eT)              (no max subtraction: |logit| <= ~70, fp32-safe)
    Z[t] = sum_s expET[s,t]      (PE ones-matmul -> broadcast across partitions)
Key algebraic folding: heads only enter the output through W1 (DFC1=128 rows),
so the huge Wv (C x C) conv and o = v @ attn (each 2.1 GF/bh) collapse into
128-channel products:
    vW1T[s,j]  = (x_b.T @ (W1 @ Wv_h).T)[s,j]          (T, 128)
    oW1raw[j,t]= sum_s vW1T[s,j] expET[s,t]            (128, T)
    fc1[j,t]   = relu(gamma_h * oW1raw[j,t]/Z[t] + xW1[b][j,t] + b1eff_h[j])
        where xW1 = W1 @ x_b, b1eff = b1 + gamma_h * (W1 @ bv_h)
        (softmax rows sum to 1 => v-bias passes through as a constant)
    out2[d,t]  = relu(W2 @ fc1 + b2)
    final[b, 8*d + h, t] = out2[d,t] + x[b, 8*d + h, t]

Sharding: data parallel - core i computes batch b=i entirely (all 8 heads).

Channel permutation: contraction over C is order-independent, so x lives in
SBUF as x'[p, ci] = x[8p + ci] (weights retiled to match). Then the residual
rows for head h (channels h, h+8, ..., h+8*127) are exactly x'[:, h, :] - a
free view of the f16 matmul copy of x, so the f32 x is never loaded at all.

DMA model (measured): every dma_start costs ~0.6us of sequencer issue time on
its queue (DIRECT2D, serialized per engine; only SP=nc.sync and ACT=nc.scalar
have HWDGE), and one dma_start's transfer runs on ~one ring (~60 GB/s), so
parallelism needs multiple descriptors. Hence: x as 16 per-(ci,half)
descriptors split across sync+scalar, per-head scalars packed into ONE (P,33)
descriptor, wqk split k/q per head with a 2-head lead, outputs on sync.

Dtypes: the logit path (QK convs + k.T@q) -> float16 (exp() amplifies absolute
logit error). Post-softmax path -> bfloat16 (exp outputs reach ~e^70). vW1T
(x.T @ (W1 Wv).T) runs in fp8e4m3 with DoubleRow (2 channel-tiles per matmul):
w1wv is pre-scaled x16 into fp8 range, compensated via gamma/16; quantization
noise lands behind gamma (~0.1) and two FC layers -> ~5e-3 of output absmax.
PSUM accumulation is fp32 throughout.

Software pipeline (PE executes its queue in order; ACT/DVE are strict FIFO):
    step i emits:  S1(c_i) eT+exp | S2(c_{i-2}) oW1+Z | S3(c_{i-2}) normalize
                   | S4+S5(c_{i-3}) FC2+store
The 2-step S1->S2 lag means every chunk's exps have ~2 chunk-periods of ACT
headroom, so the PE never waits on the exp chain - including at the drain,
where the final chunks also taper (512,256,128,128) to shrink chain latency.
Warm-up dummy matmuls on a memset tile cover the initial input-DMA window and
release the HAM clock gate before real work arrives. Residual adds run on the
otherwise idle GpSimd engine (a f16-operand ADD is a DVE slow path); the
final two chunks' run on DVE, with stores split across both DGE queues.
"""

import numpy as np

B, C, T, H, P = 8, 1024, 1024, 8, 128
CT = C // P      # 8 contraction k-tiles over channels
ST = T // P      # 8 s-tiles (softmax/partition dim)
TCW = 512        # t-chunk width (matmul moving free dim)
NWARM = 10       # warm-up dummy matmuls covering the startup DMA window
WVS = 16.0       # host pre-scale on w1wv for fp8 range

_module_cache = {}


def _build_module():
    from contextlib import ExitStack

    import concourse.bacc as bacc
    import concourse.bass as bass
    import concourse.mybir as mybir
    import concourse.tile as tile

    f32 = mybir.dt.float32
    ldt = mybir.dt.float16
    mdt = mybir.dt.bfloat16
    f8 = mybir.dt.float8e4
    AF = mybir.ActivationFunctionType
    ALU = mybir.AluOpType
    DR = mybir.MatmulPerfMode.DoubleRow

    nc = bacc.Bacc(trn_type="TRN2", name="mha_dp")

    # x16[p, ci, t] = x[8p + ci, t] (host-side reshape of the f16 cast)
    x16_d = nc.dram_tensor("x16", (P, CT, T), ldt, kind="ExternalInput")
    x8_d = nc.dram_tensor("x8", (P, CT, T), f8, kind="ExternalInput")
    # wqk[h, p, ci, 0:128]=Wq[h,:,8p+ci], 128:256 for Wk
    wqk_d = nc.dram_tensor("wqk", (H, P, CT, 256), ldt, kind="ExternalInput")
    w1wv_d = nc.dram_tensor("w1wv", (P, CT, H * P), f8, kind="ExternalInput")
    w1t_d = nc.dram_tensor("w1t", (P, CT, P), ldt, kind="ExternalInput")
    # w2o[:, 0:128] = W2.T, w2o[:, 128:256] = ones
    w2o_d = nc.dram_tensor("w2o", (P, 2 * P), mdt, kind="ExternalInput")
    # bias[:, h]=bq[h]; [:, H+h]=bk[h]; [:, 2H+h]=b1e[h]; [:, 3H+h]=gamma[h]/WVS; [:, 4H]=b2
    bias_d = nc.dram_tensor("bias", (P, 4 * H + 1), f32, kind="ExternalInput")
    out_d = nc.dram_tensor("out", (C, T), ldt, kind="ExternalOutput")

    def mm(ps, lhsT, rhs, start, stop, **kw):
        nc.tensor.matmul(ps, lhsT, rhs, start=start, stop=stop, **kw)

    with tile.TileContext(nc) as tc, ExitStack() as ctx:
        consts = ctx.enter_context(tc.tile_pool(name="consts", bufs=1))
        psA = ctx.enter_context(tc.tile_pool(name="psA", bufs=4, space="PSUM"))
        psB = ctx.enter_context(tc.tile_pool(name="psB", bufs=1, space="PSUM"))

        wqkp = ctx.enter_context(tc.tile_pool(name="wqkp", bufs=3))
        qkp = ctx.enter_context(tc.tile_pool(name="qkp", bufs=3))
        expp = ctx.enter_context(tc.tile_pool(name="expp", bufs=6))
        hbuf = ctx.enter_context(tc.tile_pool(name="hbuf", bufs=2))
        outp = ctx.enter_context(tc.tile_pool(name="outp", bufs=3))

        # ---------------- startup DMAs: few LARGE dma_starts (each costs
        # ~0.6us of sequencer issue), split across the two HWDGE queues so
        # the first-needed bytes (w1t + x first halves) land by ~4.5us.
        # Descriptors of all dma_starts spread over all 16 rings (~360GB/s).
        dumm_sb = consts.tile([P, 128 + TCW], ldt, name="dumm_sb")
        nc.vector.memset(dumm_sb[:], 0.25)

        w1t_sb = consts.tile([P, CT, P], ldt, name="w1t_sb")
        xl_sb = consts.tile([P, CT, T], ldt, name="xl_sb")
        w1wv_sb = consts.tile([P, CT, H * P], f8, name="w1wv_sb")
        x8_sb = consts.tile([P, CT, T], f8, name="x8_sb")
        bias_sb = consts.tile([P, 4 * H + 1], f32, name="bias_sb")
        w2o_sb = consts.tile([P, 2 * P], mdt, name="w2o_sb")

        head_state = {}

        def emit_head_dmas(h, eng=None):
            wqk_sb = wqkp.tile([P, CT, 256], ldt, name="wqk_sb", tag="wqk")
            (eng or nc.sync).dma_start(out=wqk_sb[:], in_=wqk_d[h][:])
            head_state[h] = dict(
                wqk=wqk_sb,
                bq=bias_sb[:, h : h + 1],
                bk=bias_sb[:, H + h : H + h + 1],
                b1e=bias_sb[:, 2 * H + h : 2 * H + h + 1],
                gam=bias_sb[:, 3 * H + h : 3 * H + h + 1],
                xres=xl_sb[:, h, :],
            )

        # sync (SP) queue: first-matmul critical path, then head 0/1 weights
        nc.sync.dma_start(out=w1t_sb[:], in_=w1t_d[:])
        nc.sync.dma_start(out=xl_sb[:, 0:4, 0:512], in_=x16_d[:, 0:4, 0:512])
        emit_head_dmas(0)
        nc.sync.dma_start(out=xl_sb[:, 0:4, 512:1024], in_=x16_d[:, 0:4, 512:1024])
        emit_head_dmas(1)
        nc.sync.dma_start(out=x8_sb[:, 0:4, :], in_=x8_d[:, 0:4, :])
        nc.sync.dma_start(out=x8_sb[:, 4:8, :], in_=x8_d[:, 4:8, :])

        # scalar (ACT) queue: other x half-tiles + fp8 weights + scalars
        nc.scalar.dma_start(out=xl_sb[:, 4:8, 0:512], in_=x16_d[:, 4:8, 0:512])
        nc.scalar.dma_start(out=bias_sb, in_=bias_d[:])
        nc.scalar.dma_start(out=xl_sb[:, 4:8, 512:1024], in_=x16_d[:, 4:8, 512:1024])
        nc.scalar.dma_start(out=w2o_sb, in_=w2o_d[:])
        nc.scalar.dma_start(out=w1wv_sb[:, :, 0:512], in_=w1wv_d[:, :, 0:512])
        nc.scalar.dma_start(out=w1wv_sb[:, :, 512:1024], in_=w1wv_d[:, :, 512:1024])

        w2t_sb = w2o_sb[:, 0:P]
        ones_sb = w2o_sb[:, P : 2 * P]
        b2_sb = bias_sb[:, 4 * H : 4 * H + 1]
        xw1_sb = consts.tile([P, T], f32, name="xw1_sb")
        vw1t_sb = consts.tile([P, ST, H * P], mdt, name="vw1t_sb")

        # ---------------- warm-up: dummy matmuls on the memset tile release
        # the HAM clock gate (~3.4us of PE busy) while input DMAs stream in.
        ps_w = psB.tile([P, TCW], f32, name="ps_w", tag="zf")
        for i in range(NWARM):
            mm(ps_w, dumm_sb[:, 0:P], dumm_sb[:, P : P + TCW], True, True)

        chunk_state = {}

        def emit_head_qk(h):
            hs = head_state[h]
            wqk_sb = hs["wqk"]
            q_sb = qkp.tile([P, T], ldt, name="q_sb", tag="q")
            k_sb = qkp.tile([P, T], ldt, name="k_sb", tag="k")
            out_sb = outp.tile([P, T], ldt, name="out_sb", tag="ob")
            # K then Q per t-half (first two groups only need the first x
            # half); bias-adds hide under later MM groups
            for t2 in range(2):
                tsl = slice(t2 * 512, (t2 + 1) * 512)
                ps_k = psB.tile([P, TCW], f32, name="ps_k", tag="qk", bufs=2)
                for ci in range(CT):
                    mm(ps_k, wqk_sb[:, ci, P : 2 * P], xl_sb[:, ci, tsl], ci == 0, ci == CT - 1)
                nc.vector.tensor_scalar_add(out=k_sb[:, tsl], in0=ps_k, scalar1=hs["bk"])
                ps_q = psB.tile([P, TCW], f32, name="ps_q", tag="qk", bufs=2)
                for ci in range(CT):
                    mm(ps_q, wqk_sb[:, ci, 0:P], xl_sb[:, ci, tsl], ci == 0, ci == CT - 1)
                nc.vector.tensor_scalar_add(out=q_sb[:, tsl], in0=ps_q, scalar1=hs["bq"])
            hs["q"] = q_sb
            hs["k"] = k_sb
            hs["out"] = out_sb

        def emit_s1_half(c, first):
            h, t_off, t_w = c
            hs = head_state[h]
            tsl = slice(t_off, t_off + t_w)
            if first:
                et_sb = expp.tile([P, ST, TCW], mdt, name="et_sb", tag="exp")
                chunk_state[c] = dict(et=et_sb)
            else:
                et_sb = chunk_state[c]["et"]
            rng = range(0, ST // 2) if first else range(ST // 2, ST)
            for si in rng:
                ps_e = psA.tile([P, TCW], f32, name="ps_e", tag="acc")
                mm(ps_e[:, :t_w], hs["k"][:, si * P : (si + 1) * P], hs["q"][:, tsl], True, True)
                nc.scalar.activation(out=et_sb[:, si, :t_w], in_=ps_e[:, :t_w], func=AF.Exp)

        def emit_s2_mm(c):
            h, t_off, t_w = c
            cs = chunk_state[c]
            et_sb = cs["et"]
            ps_o = psB.tile([P, TCW], f32, name="ps_o", tag="oo")
            for si in range(ST):
                mm(
                    ps_o[:, :t_w],
                    vw1t_sb[:, si, h * P : (h + 1) * P],
                    et_sb[:, si, :t_w],
                    si == 0,
                    si == ST - 1,
                )
            cs["ps_o"] = ps_o

        def emit_s2_s3(c):
            h, t_off, t_w = c
            hs = head_state[h]
            cs = chunk_state[c]
            tsl = slice(t_off, t_off + t_w)
            et_sb = cs["et"]
            ps_o = cs["ps_o"]
            # Z: tree-sum the 8 s-tiles on DVE (free-dim adds), then a single
            # ones-matmul for the partition reduction + broadcast.
            r1 = hbuf.tile([P, 4, TCW], mdt, name="r1", tag="r1")
            nc.vector.tensor_add(r1[:, :, :t_w], et_sb[:, 0:4, :t_w], et_sb[:, 4:8, :t_w])
            r2 = hbuf.tile([P, 2, TCW], mdt, name="r2", tag="r2")
            nc.vector.tensor_add(r2[:, :, :t_w], r1[:, 0:2, :t_w], r1[:, 2:4, :t_w])
            etsum = hbuf.tile([P, TCW], mdt, name="etsum", tag="etsum")
            nc.vector.tensor_add(etsum[:, :t_w], r2[:, 0, :t_w], r2[:, 1, :t_w])
            ps_z = psB.tile([P, TCW], f32, name="ps_z", tag="zf")
            mm(ps_z[:, :t_w], ones_sb, etsum[:, :t_w], True, True)
            # fc1 = relu(gamma * oW1/Z + xW1 + b1eff): DVE chain, relu on ACT
            izg = hbuf.tile([P, TCW], f32, name="izg", tag="izg")
            nc.vector.reciprocal_approx_fast(out=izg[:, :t_w], in_=ps_z[:, :t_w])
            t1 = hbuf.tile([P, TCW], f32, name="t1", tag="t1")
            nc.vector.scalar_tensor_tensor(
                out=t1[:, :t_w], in0=ps_o[:, :t_w], scalar=hs["gam"], in1=izg[:, :t_w],
                op0=ALU.mult, op1=ALU.mult,
            )
            t2t = hbuf.tile([P, TCW], f32, name="t2t", tag="t2t")
            nc.vector.scalar_tensor_tensor(
                out=t2t[:, :t_w], in0=t1[:, :t_w], scalar=hs["b1e"], in1=xw1_sb[:, tsl],
                op0=ALU.add, op1=ALU.add,
            )
            fc1 = hbuf.tile([P, TCW], mdt, name="fc1", tag="fc1")
            nc.scalar.activation(out=fc1[:, :t_w], in_=t2t[:, :t_w], func=AF.Relu)
            cs["fc1"] = fc1

        def emit_s4_s5(c, dve_add=False, dma_eng=None):
            h, t_off, t_w = c
            hs = head_state[h]
            cs = chunk_state[c]
            tsl = slice(t_off, t_off + t_w)
            ps_f = psB.tile([P, TCW], f32, name="ps_f", tag="zf")
            mm(ps_f[:, :t_w], w2t_sb, cs["fc1"][:, :t_w], True, True)
            ot = hbuf.tile([P, TCW], f32, name="ot", tag="ot")
            nc.scalar.activation(out=ot[:, :t_w], in_=ps_f[:, :t_w], func=AF.Relu, bias=b2_sb)
            adder = nc.vector if dve_add else nc.gpsimd
            adder.tensor_add(hs["out"][:, tsl], ot[:, :t_w], hs["xres"][:, tsl])
            out_all = out_d[:]
            (dma_eng or nc.sync).dma_start(
                out=bass.AP(
                    tensor=out_all.tensor,
                    offset=h * T + t_off,
                    ap=[[H * T, P], [1, t_w]],
                ),
                in_=hs["out"][:, tsl],
            )

        # ---- phase A: xW1 chunk 0 (needs only w1t + first x half), head-0
        # QK (first two groups need only the first x half), xW1 chunk 1,
        # head-1 QK, then vW1T once the fp8 pair has streamed in.
        def emit_xw1(t2):
            tsl = slice(t2 * 512, (t2 + 1) * 512)
            ps_x = psA.tile([P, TCW], f32, name="ps_x", tag="acc")
            for ci in range(CT):
                mm(ps_x, w1t_sb[:, ci, :], xl_sb[:, ci, tsl], ci == 0, ci == CT - 1)
            nc.scalar.activation(out=xw1_sb[:, tsl], in_=ps_x, func=AF.Copy)

        chunks = [(h, t2 * 512, 512) for h in range(H - 1) for t2 in range(2)]
        # taper the final chunks: chain latency scales with width
        chunks += [(7, 0, 512), (7, 512, 256), (7, 768, 128), (7, 896, 128)]
        N = len(chunks)

        emit_xw1(0)
        emit_head_qk(0)
        emit_xw1(1)
        emit_head_qk(1)

        # vW1T = x.T @ (W1 Wv).T for all heads, fp8 DoubleRow: 2 channel
        # tiles contract per matmul ([P, 2, .] pair slices on both operands).
        # jh-major so the first w1wv half alone unblocks the first 8 groups.
        for jh in range(2):
            jsl = slice(jh * 512, (jh + 1) * 512)
            for si in range(ST):
                ps_v = psA.tile([P, TCW], f32, name="ps_v", tag="acc")
                for a in range(CT // 2):
                    mm(
                        ps_v,
                        x8_sb[:, 2 * a : 2 * a + 2, si * P : (si + 1) * P],
                        w1wv_sb[:, 2 * a : 2 * a + 2, jsl],
                        a == 0,
                        a == CT // 2 - 1,
                        perf_mode=DR,
                    )
                nc.scalar.activation(out=vw1t_sb[:, si, jsl], in_=ps_v, func=AF.Copy)

        # Steady-state iteration (S4 BEFORE S3: FC2(c-3)'s psum bank was
        # freed by recip(c-3) a full period ago, while Z(c-2) -- emitted
        # last -- keeps the in-order PE from stalling on the DVE chain).
        # Tail: S1 of the last two (128-wide) chunks is pulled one
        # iteration early (expp bufs=6 holds them) so their exp chains
        # complete before the drain's S2/S3/S4 cascade needs them.
        for i, c in enumerate(chunks):
            h, t_off, _ = c
            if t_off == 0:
                if h <= 5:
                    emit_head_dmas(h + 2)
                if 1 <= h <= 6:
                    emit_head_qk(h + 1)
            if i <= N - 3:
                emit_s1_half(c, True)
            if i >= 2:
                emit_s2_mm(chunks[i - 2])
            if i <= N - 3:
                emit_s1_half(c, False)
            if i >= 3:
                emit_s4_s5(chunks[i - 3])
            if i >= 2:
                emit_s2_s3(chunks[i - 2])
            if i == N - 3:
                emit_s1_half(chunks[N - 2], True)
                emit_s1_half(chunks[N - 2], False)
            if i == N - 2:
                emit_s1_half(chunks[N - 1], True)
                emit_s1_half(chunks[N - 1], False)
        emit_s2_mm(chunks[N - 2])
        emit_s4_s5(chunks[N - 3])
        emit_s2_s3(chunks[N - 2])
        emit_s2_mm(chunks[N - 1])
        emit_s2_s3(chunks[N - 1])
        emit_s4_s5(chunks[N - 2], dve_add=True, dma_eng=nc.scalar)
        emit_s4_s5(chunks[N - 1], dve_add=True, dma_eng=nc.sync)

    nc.compile()
    return nc


def _prepare_inputs(inputs):
    import ml_dtypes

    f16 = np.float16
    bf16 = ml_dtypes.bfloat16
    f8 = ml_dtypes.float8_e4m3fn

    x = np.ascontiguousarray(np.asarray(inputs["x"], dtype=np.float32))
    Wq = np.asarray(inputs["Wq"], dtype=np.float32)
    bq = np.asarray(inputs["bq"], dtype=np.float32)
    Wk = np.asarray(inputs["Wk"], dtype=np.float32)
    bk = np.asarray(inputs["bk"], dtype=np.float32)
    Wv = np.asarray(inputs["Wv"], dtype=np.float32)
    bv = np.asarray(inputs["bv"], dtype=np.float32)
    gamma = np.asarray(inputs["gamma"], dtype=np.float32)
    W1 = np.asarray(inputs["W1"], dtype=np.float32)
    b1 = np.asarray(inputs["b1"], dtype=np.float32)
    W2 = np.asarray(inputs["W2"], dtype=np.float32)
    b2 = np.asarray(inputs["b2"], dtype=np.float32)

    # channel permutation c = 8p + ci: plain reshape of the (C, x) transposes
    # wqk[h, p, ci, 0:128] = Wq[h, :, 8p+ci]; 128:256 for Wk
    wqk = np.empty((H, P, CT, 256), dtype=np.float32)
    for h in range(H):
        wqk[h, :, :, 0:P] = Wq[h].T.reshape(P, CT, P)
        wqk[h, :, :, P : 2 * P] = Wk[h].T.reshape(P, CT, P)

    # w1wv[p, ci, h*128+j] = WVS * (W1 @ Wv[h]).T[8p+ci, j], fp8 with a x16
    # range pre-scale; compensated by shipping gamma/WVS
    w1wvT = np.concatenate([(W1 @ Wv[h]).T for h in range(H)], axis=1)  # (C, H*128)
    w1wv = w1wvT.reshape(P, CT, H * P) * WVS

    w1t = W1.T.reshape(P, CT, P)
    w2o = np.concatenate([W2.T, np.ones((P, P), dtype=np.float32)], axis=1)

    b1v = bv @ W1.T  # (H, P): b1v[h] = W1 @ bv[h]
    b1e = b1[None, :] + gamma[:, None] * b1v  # (H, P)
    # bias[:, h]=bq[h]; [:, H+h]=bk[h]; [:, 2H+h]=b1e[h]; [:, 3H+h]=gam/WVS; [:, 4H]=b2
    bias = np.empty((P, 4 * H + 1), dtype=np.float32)
    bias[:, 0:H] = bq.T
    bias[:, H : 2 * H] = bk.T
    bias[:, 2 * H : 3 * H] = b1e.T
    bias[:, 3 * H : 4 * H] = np.tile(gamma[None, :] / WVS, (P, 1))
    bias[:, 4 * H] = b2

    shared = {
        "wqk": np.ascontiguousarray(wqk.astype(f16)),
        "w1wv": np.ascontiguousarray(w1wv.astype(f8)),
        "w1t": np.ascontiguousarray(w1t.astype(f16)),
        "w2o": np.ascontiguousarray(w2o.astype(bf16)),
        "bias": bias,
    }
    in_maps = []
    for b in range(B):
        m = dict(shared)
        xr = x[b].reshape(P, CT, T)
        m["x16"] = np.ascontiguousarray(xr.astype(f16))
        m["x8"] = np.ascontiguousarray(xr.astype(f8))
        in_maps.append(m)
    return in_maps


def kernel(**inputs):
    from concourse.bass_utils import run_bass_kernel_spmd

    if "nc" not in _module_cache:
        _module_cache["nc"] = _build_module()
    nc = _module_cache["nc"]

    in_maps = _prepare_inputs(inputs)
    res = run_bass_kernel_spmd(nc, in_maps, core_ids=list(range(B)))
    out = np.stack([res.results[b]["out"] for b in range(B)], axis=0)
    return out.astype(np.float32)



# revision 9
# speedup vs baseline: 1.0535x; 1.0535x over previous
"""Trainium2 Bass kernel for nn_MultiHeadAttention_9491877724818.

Math (per batch b, head h), reformulated from the reference:
    q = Wq_h @ x_b + bq          (128, T)
    k = Wk_h @ x_b + bk          (128, T)
    eT[s,t] = (k.T @ q)[s,t]     == energy[t,s]; softmax over s (partition dim)
    expET = exp(eT)              (no max subtraction: |logit| <= ~70, fp32-safe)
    Z[t] = sum_s expET[s,t]      (PE ones-matmul -> broadcast across partitions)
Key algebraic folding: heads only enter the output through W1 (DFC1=128 rows),
so the huge Wv (C x C) conv and o = v @ attn (each 2.1 GF/bh) collapse into
128-channel products:
    vW1T[s,j]  = (x_b.T @ (W1 @ Wv_h).T)[s,j]          (T, 128)
    oW1raw[j,t]= sum_s vW1T[s,j] expET[s,t]            (128, T)
    fc1[j,t]   = relu(gamma_h * oW1raw[j,t]/Z[t] + xW1[b][j,t] + b1eff_h[j])
        where xW1 = W1 @ x_b, b1eff = b1 + gamma_h * (W1 @ bv_h)
        (softmax rows sum to 1 => v-bias passes through as a constant)
    out2[d,t]  = relu(W2 @ fc1 + b2)
    final[b, 8*d + h, t] = out2[d,t] + x[b, 8*d + h, t]

Sharding: data parallel - core i computes batch b=i entirely (all 8 heads).

Channel permutation: contraction over C is order-independent, so x lives in
SBUF as x'[p, ci] = x[8p + ci] (weights retiled to match). Then the residual
rows for head h (channels h, h+8, ..., h+8*127) are exactly x'[:, h, :] - a
free view of the f16 matmul copy of x, so the f32 x is never loaded at all.

DMA model (measured): every dma_start costs ~0.6us of sequencer issue time on
its queue (DIRECT2D, serialized per engine; only SP=nc.sync and ACT=nc.scalar
have HWDGE), and one dma_start's transfer runs on ~one ring (~60 GB/s), so
parallelism needs multiple descriptors. Hence: x as 16 per-(ci,half)
descriptors split across sync+scalar, per-head scalars packed into ONE (P,33)
descriptor, wqk split k/q per head with a 2-head lead, outputs on sync.

Dtypes: the logit path (QK convs + k.T@q) -> float16 (exp() amplifies absolute
logit error). Post-softmax path -> bfloat16 (exp outputs reach ~e^70). vW1T
(x.T @ (W1 Wv).T) runs in fp8e4m3 with DoubleRow (2 channel-tiles per matmul):
w1wv is pre-scaled x16 into fp8 range, compensated via gamma/16; quantization
noise lands behind gamma (~0.1) and two FC layers -> ~5e-3 of output absmax.
PSUM accumulation is fp32 throughout.

Software pipeline (PE executes its queue in order; ACT/DVE are strict FIFO):
    step i emits:  S1(c_i) eT+exp | S2(c_{i-2}) oW1+Z | S3(c_{i-2}) normalize
                   | S4+S5(c_{i-3}) FC2+store
The 2-step S1->S2 lag means every chunk's exps have ~2 chunk-periods of ACT
headroom, so the PE never waits on the exp chain - including at the drain,
where the final chunks also taper (512,256,128,128) to shrink chain latency.
Warm-up dummy matmuls on a memset tile cover the initial input-DMA window and
release the HAM clock gate before real work arrives. Residual adds run on the
otherwise idle GpSimd engine (a f16-operand ADD is a DVE slow path); the
final two chunks' run on DVE, with stores split across both DGE queues.
"""

import numpy as np

B, C, T, H, P = 8, 1024, 1024, 8, 128
CT = C // P      # 8 contraction k-tiles over channels
ST = T // P      # 8 s-tiles (softmax/partition dim)
TCW = 512        # t-chunk width (matmul moving free dim)
NWARM = 10       # warm-up dummy matmuls covering the startup DMA window
WVS = 16.0       # host pre-scale on w1wv for fp8 range

_module_cache = {}


def _build_module():
    from contextlib import ExitStack

    import concourse.bacc as bacc
    import concourse.bass as bass
    import concourse.mybir as mybir
    import concourse.tile as tile

    f32 = mybir.dt.float32
    ldt = mybir.dt.float16
    mdt = mybir.dt.bfloat16
    f8 = mybir.dt.float8e4
    AF = mybir.ActivationFunctionType
    ALU = mybir.AluOpType
    DR = mybir.MatmulPerfMode.DoubleRow

    nc = bacc.Bacc(trn_type="TRN2", name="mha_dp")

    # x16[p, ci, t] = x[8p + ci, t] (host-side reshape of the f16 cast)
    x16_d = nc.dram_tensor("x16", (P, CT, T), ldt, kind="ExternalInput")
    x8_d = nc.dram_tensor("x8", (P, CT, T), f8, kind="ExternalInput")
    # wqk[h, p, ci, 0:128]=Wq[h,:,8p+ci], 128:256 for Wk
    wqk_d = nc.dram_tensor("wqk", (H, P, CT, 256), ldt, kind="ExternalInput")
    w1wv_d = nc.dram_tensor("w1wv", (P, CT, H * P), f8, kind="ExternalInput")
    w1t_d = nc.dram_tensor("w1t", (P, CT, P), ldt, kind="ExternalInput")
    # w2o[:, 0:128] = W2.T, w2o[:, 128:256] = ones
    w2o_d = nc.dram_tensor("w2o", (P, 2 * P), mdt, kind="ExternalInput")
    # bias[:, h]=bq[h]; [:, H+h]=bk[h]; [:, 2H+h]=b1e[h]; [:, 3H+h]=gamma[h]/WVS; [:, 4H]=b2
    bias_d = nc.dram_tensor("bias", (P, 4 * H + 1), f32, kind="ExternalInput")
    out_d = nc.dram_tensor("out", (C, T), ldt, kind="ExternalOutput")

    def mm(ps, lhsT, rhs, start, stop, **kw):
        nc.tensor.matmul(ps, lhsT, rhs, start=start, stop=stop, **kw)

    with tile.TileContext(nc) as tc, ExitStack() as ctx:
        consts = ctx.enter_context(tc.tile_pool(name="consts", bufs=1))
        psA = ctx.enter_context(tc.tile_pool(name="psA", bufs=4, space="PSUM"))
        psB = ctx.enter_context(tc.tile_pool(name="psB", bufs=1, space="PSUM"))

        wqkp = ctx.enter_context(tc.tile_pool(name="wqkp", bufs=3))
        qkp = ctx.enter_context(tc.tile_pool(name="qkp", bufs=3))
        expp = ctx.enter_context(tc.tile_pool(name="expp", bufs=6))
        hbuf = ctx.enter_context(tc.tile_pool(name="hbuf", bufs=2))
        outp = ctx.enter_context(tc.tile_pool(name="outp", bufs=3))

        # ---------------- startup DMAs: few LARGE dma_starts (each costs
        # ~0.6us of sequencer issue), split across the two HWDGE queues so
        # the first-needed bytes (w1t + x first halves) land by ~4.5us.
        # Descriptors of all dma_starts spread over all 16 rings (~360GB/s).
        dumm_sb = consts.tile([P, 128 + TCW], ldt, name="dumm_sb")
        nc.vector.memset(dumm_sb[:], 0.25)

        w1t_sb = consts.tile([P, CT, P], ldt, name="w1t_sb")
        # xl/w1wv split into half tiles: consumers of one half must not
        # inherit DMA dependencies on the other (tile-granular hazards)
        xl_lo = consts.tile([P, CT, 512], ldt, name="xl_lo")
        xl_hi = consts.tile([P, CT, 512], ldt, name="xl_hi")
        w1wv0_sb = consts.tile([P, CT, 512], f8, name="w1wv0_sb")
        w1wv1_sb = consts.tile([P, CT, 512], f8, name="w1wv1_sb")
        x8_sb = consts.tile([P, CT, T], f8, name="x8_sb")
        bias_sb = consts.tile([P, 4 * H + 1], f32, name="bias_sb")
        w2o_sb = consts.tile([P, 2 * P], mdt, name="w2o_sb")

        head_state = {}

        def emit_head_dmas(h, eng=None):
            wqk_sb = wqkp.tile([P, CT, 256], ldt, name="wqk_sb", tag="wqk")
            (eng or nc.sync).dma_start(out=wqk_sb[:], in_=wqk_d[h][:])
            head_state[h] = dict(
                wqk=wqk_sb,
                bq=bias_sb[:, h : h + 1],
                bk=bias_sb[:, H + h : H + h + 1],
                b1e=bias_sb[:, 2 * H + h : 2 * H + h + 1],
                gam=bias_sb[:, 3 * H + h : 3 * H + h + 1],
                xres_lo=xl_lo[:, h, :],
                xres_hi=xl_hi[:, h, :],
            )

        # sync (SP) queue: first-matmul critical path, then head 0/1 weights
        nc.sync.dma_start(out=w1t_sb[:], in_=w1t_d[:])
        nc.sync.dma_start(out=xl_lo[:, 0:4, :], in_=x16_d[:, 0:4, 0:512])
        emit_head_dmas(0)
        nc.sync.dma_start(out=xl_hi[:, 0:4, :], in_=x16_d[:, 0:4, 512:1024])
        emit_head_dmas(1)
        nc.sync.dma_start(out=x8_sb[:, 0:4, :], in_=x8_d[:, 0:4, :])
        nc.sync.dma_start(out=x8_sb[:, 4:8, :], in_=x8_d[:, 4:8, :])

        # scalar (ACT) queue: other x half-tiles + fp8 weights + scalars
        nc.scalar.dma_start(out=xl_lo[:, 4:8, :], in_=x16_d[:, 4:8, 0:512])
        nc.scalar.dma_start(out=bias_sb, in_=bias_d[:])
        nc.scalar.dma_start(out=xl_hi[:, 4:8, :], in_=x16_d[:, 4:8, 512:1024])
        nc.scalar.dma_start(out=w2o_sb, in_=w2o_d[:])
        nc.scalar.dma_start(out=w1wv0_sb[:], in_=w1wv_d[:, :, 0:512])
        nc.scalar.dma_start(out=w1wv1_sb[:], in_=w1wv_d[:, :, 512:1024])

        w2t_sb = w2o_sb[:, 0:P]
        ones_sb = w2o_sb[:, P : 2 * P]
        b2_sb = bias_sb[:, 4 * H : 4 * H + 1]
        xw1_sb = consts.tile([P, T], f32, name="xw1_sb")
        vw1t_sb = consts.tile([P, ST, H * P], mdt, name="vw1t_sb")

        # ---------------- warm-up: dummy matmuls on the memset tile release
        # the HAM clock gate (~3.4us of PE busy) while input DMAs stream in.
        ps_w = psB.tile([P, TCW], f32, name="ps_w", tag="zf")
        for i in range(NWARM):
            mm(ps_w, dumm_sb[:, 0:P], dumm_sb[:, P : P + TCW], True, True)

        chunk_state = {}

        def emit_head_qk(h):
            hs = head_state[h]
            wqk_sb = hs["wqk"]
            q_sb = qkp.tile([P, T], ldt, name="q_sb", tag="q")
            k_sb = qkp.tile([P, T], ldt, name="k_sb", tag="k")
            out_sb = outp.tile([P, T], ldt, name="out_sb", tag="ob")
            # K then Q per t-half (first two groups only need the first x
            # half); bias-adds hide under later MM groups
            for t2 in range(2):
                tsl = slice(t2 * 512, (t2 + 1) * 512)
                xh = xl_lo if t2 == 0 else xl_hi
                ps_k = psB.tile([P, TCW], f32, name="ps_k", tag="qk", bufs=2)
                for ci in range(CT):
                    mm(ps_k, wqk_sb[:, ci, P : 2 * P], xh[:, ci, :], ci == 0, ci == CT - 1)
                nc.vector.tensor_scalar_add(out=k_sb[:, tsl], in0=ps_k, scalar1=hs["bk"])
                ps_q = psB.tile([P, TCW], f32, name="ps_q", tag="qk", bufs=2)
                for ci in range(CT):
                    mm(ps_q, wqk_sb[:, ci, 0:P], xh[:, ci, :], ci == 0, ci == CT - 1)
                nc.vector.tensor_scalar_add(out=q_sb[:, tsl], in0=ps_q, scalar1=hs["bq"])
            hs["q"] = q_sb
            hs["k"] = k_sb
            hs["out"] = out_sb

        def emit_s1_half(c, first):
            h, t_off, t_w = c
            hs = head_state[h]
            tsl = slice(t_off, t_off + t_w)
            if first:
                et_sb = expp.tile([P, ST, TCW], mdt, name="et_sb", tag="exp")
                chunk_state[c] = dict(et=et_sb)
            else:
                et_sb = chunk_state[c]["et"]
            rng = range(0, ST // 2) if first else range(ST // 2, ST)
            for si in rng:
                ps_e = psA.tile([P, TCW], f32, name="ps_e", tag="acc")
                mm(ps_e[:, :t_w], hs["k"][:, si * P : (si + 1) * P], hs["q"][:, tsl], True, True)
                nc.scalar.activation(out=et_sb[:, si, :t_w], in_=ps_e[:, :t_w], func=AF.Exp)

        def emit_s2_mm(c, late=False):
            h, t_off, t_w = c
            cs = chunk_state[c]
            et_sb = cs["et"]
            # late chunks borrow the dead QK-conv psum banks: the single
            # 'oo' bank would serialize each drain S2 behind the previous
            # chunk's DVE chain
            if late:
                ps_o = psB.tile([P, TCW], f32, name="ps_o", tag="qk", bufs=2)
            else:
                ps_o = psB.tile([P, TCW], f32, name="ps_o", tag="oo")
            for si in range(ST):
                mm(
                    ps_o[:, :t_w],
                    vw1t_sb[:, si, h * P : (h + 1) * P],
                    et_sb[:, si, :t_w],
                    si == 0,
                    si == ST - 1,
                )
            cs["ps_o"] = ps_o

        def emit_s2_s3(c, late=False):
            h, t_off, t_w = c
            hs = head_state[h]
            cs = chunk_state[c]
            tsl = slice(t_off, t_off + t_w)
            et_sb = cs["et"]
            ps_o = cs["ps_o"]
            # Z: tree-sum the 8 s-tiles on DVE (free-dim adds), then a single
            # ones-matmul for the partition reduction + broadcast.
            r1 = hbuf.tile([P, 4, TCW], mdt, name="r1", tag="r1")
            nc.vector.tensor_add(r1[:, :, :t_w], et_sb[:, 0:4, :t_w], et_sb[:, 4:8, :t_w])
            r2 = hbuf.tile([P, 2, TCW], mdt, name="r2", tag="r2")
            nc.vector.tensor_add(r2[:, :, :t_w], r1[:, 0:2, :t_w], r1[:, 2:4, :t_w])
            etsum = hbuf.tile([P, TCW], mdt, name="etsum", tag="etsum")
            nc.vector.tensor_add(etsum[:, :t_w], r2[:, 0, :t_w], r2[:, 1, :t_w])
            if late:
                ps_z = psA.tile([P, TCW], f32, name="ps_z", tag="acc")
            else:
                ps_z = psB.tile([P, TCW], f32, name="ps_z", tag="zf")
            mm(ps_z[:, :t_w], ones_sb, etsum[:, :t_w], True, True)
            # fc1 = relu(gamma * oW1/Z + xW1 + b1eff): DVE chain, relu on ACT
            izg = hbuf.tile([P, TCW], f32, name="izg", tag="izg")
            nc.vector.reciprocal_approx_fast(out=izg[:, :t_w], in_=ps_z[:, :t_w])
            t1 = hbuf.tile([P, TCW], f32, name="t1", tag="t1")
            nc.vector.scalar_tensor_tensor(
                out=t1[:, :t_w], in0=ps_o[:, :t_w], scalar=hs["gam"], in1=izg[:, :t_w],
                op0=ALU.mult, op1=ALU.mult,
            )
            t2t = hbuf.tile([P, TCW], f32, name="t2t", tag="t2t")
            nc.vector.scalar_tensor_tensor(
                out=t2t[:, :t_w], in0=t1[:, :t_w], scalar=hs["b1e"], in1=xw1_sb[:, tsl],
                op0=ALU.add, op1=ALU.add,
            )
            fc1 = hbuf.tile([P, TCW], mdt, name="fc1", tag="fc1")
            nc.scalar.activation(out=fc1[:, :t_w], in_=t2t[:, :t_w], func=AF.Relu)
            cs["fc1"] = fc1

        def emit_s4_s5(c, dve_add=False, dma_eng=None, late=False):
            h, t_off, t_w = c
            hs = head_state[h]
            cs = chunk_state[c]
            tsl = slice(t_off, t_off + t_w)
            if late:
                ps_f = psA.tile([P, TCW], f32, name="ps_f", tag="acc")
            else:
                ps_f = psB.tile([P, TCW], f32, name="ps_f", tag="zf")
            mm(ps_f[:, :t_w], w2t_sb, cs["fc1"][:, :t_w], True, True)
            ot = hbuf.tile([P, TCW], f32, name="ot", tag="ot")
            nc.scalar.activation(out=ot[:, :t_w], in_=ps_f[:, :t_w], func=AF.Relu, bias=b2_sb)
            adder = nc.vector if dve_add else nc.gpsimd
            if t_off < 512:
                xres = hs["xres_lo"][:, t_off : t_off + t_w]
            else:
                xres = hs["xres_hi"][:, t_off - 512 : t_off - 512 + t_w]
            adder.tensor_add(hs["out"][:, tsl], ot[:, :t_w], xres)
            out_all = out_d[:]
            (dma_eng or nc.sync).dma_start(
                out=bass.AP(
                    tensor=out_all.tensor,
                    offset=h * T + t_off,
                    ap=[[H * T, P], [1, t_w]],
                ),
                in_=hs["out"][:, tsl],
            )

        # ---- phase A: xW1 chunk 0 (needs only w1t + first x half), head-0
        # QK (first two groups need only the first x half), xW1 chunk 1,
        # head-1 QK, then vW1T once the fp8 pair has streamed in.
        def emit_xw1(t2):
            tsl = slice(t2 * 512, (t2 + 1) * 512)
            xh = xl_lo if t2 == 0 else xl_hi
            ps_x = psA.tile([P, TCW], f32, name="ps_x", tag="acc")
            for ci in range(CT):
                mm(ps_x, w1t_sb[:, ci, :], xh[:, ci, :], ci == 0, ci == CT - 1)
            nc.scalar.activation(out=xw1_sb[:, tsl], in_=ps_x, func=AF.Copy)

        chunks = [(h, t2 * 512, 512) for h in range(H - 1) for t2 in range(2)]
        # taper the final chunks: chain latency scales with width
        chunks += [(7, 0, 512), (7, 512, 256), (7, 768, 128), (7, 896, 128)]
        N = len(chunks)

        emit_xw1(0)
        emit_head_qk(0)
        emit_xw1(1)
        emit_head_qk(1)

        # vW1T = x.T @ (W1 Wv).T for all heads, fp8 DoubleRow: 2 channel
        # tiles contract per matmul ([P, 2, .] pair slices on both operands).
        # jh-major so the first w1wv half alone unblocks the first 8 groups.
        for jh in range(2):
            jsl = slice(jh * 512, (jh + 1) * 512)
            wv = w1wv0_sb if jh == 0 else w1wv1_sb
            for si in range(ST):
                ps_v = psA.tile([P, TCW], f32, name="ps_v", tag="acc")
                for a in range(CT // 2):
                    mm(
                        ps_v,
                        x8_sb[:, 2 * a : 2 * a + 2, si * P : (si + 1) * P],
                        wv[:, 2 * a : 2 * a + 2, :],
                        a == 0,
                        a == CT // 2 - 1,
                        perf_mode=DR,
                    )
                nc.scalar.activation(out=vw1t_sb[:, si, jsl], in_=ps_v, func=AF.Copy)

        # Steady-state iteration (S4 BEFORE S3: FC2(c-3)'s psum bank was
        # freed by recip(c-3) a full period ago, while Z(c-2) -- emitted
        # last -- keeps the in-order PE from stalling on the DVE chain).
        # Tail: S1 of the last two (128-wide) chunks is pulled one
        # iteration early (expp bufs=6 holds them) so their exp chains
        # complete before the drain's S2/S3/S4 cascade needs them.
        for i, c in enumerate(chunks):
            h, t_off, _ = c
            if t_off == 0:
                if h <= 5:
                    emit_head_dmas(h + 2)
                if 1 <= h <= 6:
                    emit_head_qk(h + 1)
            if i <= N - 3:
                emit_s1_half(c, True)
            if i >= 2:
                emit_s2_mm(chunks[i - 2], late=(i - 2 >= N - 3))
            if i <= N - 3:
                emit_s1_half(c, False)
            if i >= 3:
                emit_s4_s5(chunks[i - 3])
            if i >= 2:
                emit_s2_s3(chunks[i - 2], late=(i - 2 >= N - 3))
            if i == N - 3:
                emit_s1_half(chunks[N - 2], True)
                emit_s1_half(chunks[N - 2], False)
            if i == N - 2:
                emit_s1_half(chunks[N - 1], True)
                emit_s1_half(chunks[N - 1], False)
        emit_s2_mm(chunks[N - 2], late=True)
        emit_s4_s5(chunks[N - 3], late=True)
        emit_s2_s3(chunks[N - 2], late=True)
        emit_s2_mm(chunks[N - 1], late=True)
        emit_s2_s3(chunks[N - 1], late=True)
        emit_s4_s5(chunks[N - 2], dve_add=True, dma_eng=nc.scalar, late=True)
        emit_s4_s5(chunks[N - 1], dve_add=True, dma_eng=nc.sync, late=True)

    nc.compile()
    return nc


def _prepare_inputs(inputs):
    import ml_dtypes

    f16 = np.float16
    bf16 = ml_dtypes.bfloat16
    f8 = ml_dtypes.float8_e4m3fn

    x = np.ascontiguousarray(np.asarray(inputs["x"], dtype=np.float32))
    Wq = np.asarray(inputs["Wq"], dtype=np.float32)
    bq = np.asarray(inputs["bq"], dtype=np.float32)
    Wk = np.asarray(inputs["Wk"], dtype=np.float32)
    bk = np.asarray(inputs["bk"], dtype=np.float32)
    Wv = np.asarray(inputs["Wv"], dtype=np.float32)
    bv = np.asarray(inputs["bv"], dtype=np.float32)
    gamma = np.asarray(inputs["gamma"], dtype=np.float32)
    W1 = np.asarray(inputs["W1"], dtype=np.float32)
    b1 = np.asarray(inputs["b1"], dtype=np.float32)
    W2 = np.asarray(inputs["W2"], dtype=np.float32)
    b2 = np.asarray(inputs["b2"], dtype=np.float32)

    # channel permutation c = 8p + ci: plain reshape of the (C, x) transposes
    # wqk[h, p, ci, 0:128] = Wq[h, :, 8p+ci]; 128:256 for Wk
    wqk = np.empty((H, P, CT, 256), dtype=np.float32)
    for h in range(H):
        wqk[h, :, :, 0:P] = Wq[h].T.reshape(P, CT, P)
        wqk[h, :, :, P : 2 * P] = Wk[h].T.reshape(P, CT, P)

    # w1wv[p, ci, h*128+j] = WVS * (W1 @ Wv[h]).T[8p+ci, j], fp8 with a x16
    # range pre-scale; compensated by shipping gamma/WVS
    w1wvT = np.concatenate([(W1 @ Wv[h]).T for h in range(H)], axis=1)  # (C, H*128)
    w1wv = w1wvT.reshape(P, CT, H * P) * WVS

    w1t = W1.T.reshape(P, CT, P)
    w2o = np.concatenate([W2.T, np.ones((P, P), dtype=np.float32)], axis=1)

    b1v = bv @ W1.T  # (H, P): b1v[h] = W1 @ bv[h]
    b1e = b1[None, :] + gamma[:, None] * b1v  # (H, P)
    # bias[:, h]=bq[h]; [:, H+h]=bk[h]; [:, 2H+h]=b1e[h]; [:, 3H+h]=gam/WVS; [:, 4H]=b2
    bias = np.empty((P, 4 * H + 1), dtype=np.float32)
    bias[:, 0:H] = bq.T
    bias[:, H : 2 * H] = bk.T
    bias[:, 2 * H : 3 * H] = b1e.T
    bias[:, 3 * H : 4 * H] = np.tile(gamma[None, :] / WVS, (P, 1))
    bias[:, 4 * H] = b2

    shared = {
        "wqk": np.ascontiguousarray(wqk.astype(f16)),
        "w1wv": np.ascontiguousarray(w1wv.astype(f8)),
        "w1t": np.ascontiguousarray(w1t.astype(f16)),
        "w2o": np.ascontiguousarray(w2o.astype(bf16)),
        "bias": bias,
    }
    in_maps = []
    for b in range(B):
        m = dict(shared)
        xr = x[b].reshape(P, CT, T)
        m["x16"] = np.ascontiguousarray(xr.astype(f16))
        m["x8"] = np.ascontiguousarray(xr.astype(f8))
        in_maps.append(m)
    return in_maps


def kernel(**inputs):
    from concourse.bass_utils import run_bass_kernel_spmd

    if "nc" not in _module_cache:
        _module_cache["nc"] = _build_module()
    nc = _module_cache["nc"]

    in_maps = _prepare_inputs(inputs)
    res = run_bass_kernel_spmd(nc, in_maps, core_ids=list(range(B)))
    out = np.stack([res.results[b]["out"] for b in range(B)], axis=0)
    return out.astype(np.float32)



# revision 11
# speedup vs baseline: 1.0643x; 1.0103x over previous
"""Trainium2 Bass kernel for nn_MultiHeadAttention_9491877724818.

Math (per batch b, head h), reformulated from the reference:
    q = Wq_h @ x_b + bq          (128, T)
    k = Wk_h @ x_b + bk          (128, T)
    eT[s,t] = (k.T @ q)[s,t]     == energy[t,s]; softmax over s (partition dim)
    expET = exp(eT)              (no max subtraction: |logit| <= ~70, fp32-safe)
    Z[t] = sum_s expET[s,t]      (PE ones-matmul -> broadcast across partitions)
Key algebraic folding: heads only enter the output through W1 (DFC1=128 rows),
so the huge Wv (C x C) conv and o = v @ attn (each 2.1 GF/bh) collapse into
128-channel products:
    vW1T[s,j]  = (x_b.T @ (W1 @ Wv_h).T)[s,j]          (T, 128)
    oW1raw[j,t]= sum_s vW1T[s,j] expET[s,t]            (128, T)
    fc1[j,t]   = relu(gamma_h * oW1raw[j,t]/Z[t] + xW1[b][j,t] + b1eff_h[j])
        where xW1 = W1 @ x_b, b1eff = b1 + gamma_h * (W1 @ bv_h)
        (softmax rows sum to 1 => v-bias passes through as a constant)
    out2[d,t]  = relu(W2 @ fc1 + b2)
    final[b, 8*d + h, t] = out2[d,t] + x[b, 8*d + h, t]

Sharding: data parallel - core i computes batch b=i entirely (all 8 heads).

Channel permutation: contraction over C is order-independent, so x lives in
SBUF as x'[p, ci] = x[8p + ci] (weights retiled to match). Then the residual
rows for head h (channels h, h+8, ..., h+8*127) are exactly x'[:, h, :] - a
free view of the f16 matmul copy of x, so the f32 x is never loaded at all.

DMA model (measured): every dma_start costs ~0.6us of sequencer issue time on
its queue (DIRECT2D, serialized per engine; only SP=nc.sync and ACT=nc.scalar
have HWDGE), and one dma_start's transfer runs on ~one ring (~60 GB/s), so
parallelism needs multiple descriptors. Hence: x as 16 per-(ci,half)
descriptors split across sync+scalar, per-head scalars packed into ONE (P,33)
descriptor, wqk split k/q per head with a 2-head lead, outputs on sync.

Dtypes: the logit path (QK convs + k.T@q) -> float16 (exp() amplifies absolute
logit error). Post-softmax path -> bfloat16 (exp outputs reach ~e^70). vW1T
(x.T @ (W1 Wv).T) runs in fp8e4m3 with DoubleRow (2 channel-tiles per matmul):
w1wv is pre-scaled x16 into fp8 range, compensated via gamma/16; quantization
noise lands behind gamma (~0.1) and two FC layers -> ~5e-3 of output absmax.
PSUM accumulation is fp32 throughout.

Software pipeline (PE executes its queue in order; ACT/DVE are strict FIFO):
    step i emits:  S1(c_i) eT+exp | S2(c_{i-2}) oW1+Z | S3(c_{i-2}) normalize
                   | S4+S5(c_{i-3}) FC2+store
The 2-step S1->S2 lag means every chunk's exps have ~2 chunk-periods of ACT
headroom, so the PE never waits on the exp chain - including at the drain,
where the final chunks also taper (512,256,128,128) to shrink chain latency.
Warm-up dummy matmuls on a memset tile cover the initial input-DMA window and
release the HAM clock gate before real work arrives. Residual adds run on the
otherwise idle GpSimd engine (a f16-operand ADD is a DVE slow path); the
final two chunks' run on DVE, with stores split across both DGE queues.
"""

import numpy as np

B, C, T, H, P = 8, 1024, 1024, 8, 128
CT = C // P      # 8 contraction k-tiles over channels
ST = T // P      # 8 s-tiles (softmax/partition dim)
TCW = 512        # t-chunk width (matmul moving free dim)
NWARM = 8       # warm-up dummy matmuls covering the startup DMA window
WVS = 16.0       # host pre-scale on w1wv for fp8 range

_module_cache = {}


def _build_module():
    from contextlib import ExitStack

    import concourse.bacc as bacc
    import concourse.bass as bass
    import concourse.mybir as mybir
    import concourse.tile as tile

    f32 = mybir.dt.float32
    ldt = mybir.dt.float16
    mdt = mybir.dt.bfloat16
    f8 = mybir.dt.float8e4
    AF = mybir.ActivationFunctionType
    ALU = mybir.AluOpType
    DR = mybir.MatmulPerfMode.DoubleRow

    nc = bacc.Bacc(trn_type="TRN2", name="mha_dp")

    # x16[p, ci, t] = x[8p + ci, t] (host-side reshape of the f16 cast)
    x16_d = nc.dram_tensor("x16", (P, CT, T), ldt, kind="ExternalInput")
    x8_d = nc.dram_tensor("x8", (P, CT, T), f8, kind="ExternalInput")
    # wqk[h, p, ci, 0:128]=Wq[h,:,8p+ci], 128:256 for Wk
    wqk_d = nc.dram_tensor("wqk", (H, P, CT, 256), ldt, kind="ExternalInput")
    w1wv_d = nc.dram_tensor("w1wv", (P, CT, H * P), f8, kind="ExternalInput")
    w1t_d = nc.dram_tensor("w1t", (P, CT, P), ldt, kind="ExternalInput")
    # w2o[:, 0:128] = W2.T, w2o[:, 128:256] = ones
    w2o_d = nc.dram_tensor("w2o", (P, 2 * P), mdt, kind="ExternalInput")
    # bias[:, h]=bq[h]; [:, H+h]=bk[h]; [:, 2H+h]=b1e[h]; [:, 3H+h]=gamma[h]/WVS; [:, 4H]=b2
    bias_d = nc.dram_tensor("bias", (P, 4 * H + 1), f32, kind="ExternalInput")
    out_d = nc.dram_tensor("out", (C, T), ldt, kind="ExternalOutput")

    def mm(ps, lhsT, rhs, start, stop, **kw):
        nc.tensor.matmul(ps, lhsT, rhs, start=start, stop=stop, **kw)

    with tile.TileContext(nc) as tc, ExitStack() as ctx:
        consts = ctx.enter_context(tc.tile_pool(name="consts", bufs=1))
        psA = ctx.enter_context(tc.tile_pool(name="psA", bufs=4, space="PSUM"))
        psB = ctx.enter_context(tc.tile_pool(name="psB", bufs=1, space="PSUM"))

        wqkp = ctx.enter_context(tc.tile_pool(name="wqkp", bufs=3))
        qkp = ctx.enter_context(tc.tile_pool(name="qkp", bufs=3))
        expp = ctx.enter_context(tc.tile_pool(name="expp", bufs=6))
        hbuf = ctx.enter_context(tc.tile_pool(name="hbuf", bufs=2))
        outp = ctx.enter_context(tc.tile_pool(name="outp", bufs=3))

        # ---------------- startup DMAs: few LARGE dma_starts (each costs
        # ~0.6us of sequencer issue), split across the two HWDGE queues so
        # the first-needed bytes (w1t + x first halves) land by ~4.5us.
        # Descriptors of all dma_starts spread over all 16 rings (~360GB/s).
        dumm_sb = consts.tile([P, 128 + TCW], ldt, name="dumm_sb")
        nc.vector.memset(dumm_sb[:], 0.25)

        w1t_sb = consts.tile([P, CT, P], ldt, name="w1t_sb")
        # xl/w1wv split into half tiles: consumers of one half must not
        # inherit DMA dependencies on the other (tile-granular hazards)
        xl_lo = consts.tile([P, CT, 512], ldt, name="xl_lo")
        xl_hi = consts.tile([P, CT, 512], ldt, name="xl_hi")
        w1wv0_sb = consts.tile([P, CT, 512], f8, name="w1wv0_sb")
        w1wv1_sb = consts.tile([P, CT, 512], f8, name="w1wv1_sb")
        x8_sb = consts.tile([P, CT, T], f8, name="x8_sb")
        bias_sb = consts.tile([P, 4 * H + 1], f32, name="bias_sb")
        w2o_sb = consts.tile([P, 2 * P], mdt, name="w2o_sb")

        head_state = {}

        def emit_head_dmas(h, eng=None):
            wqk_sb = wqkp.tile([P, CT, 256], ldt, name="wqk_sb", tag="wqk")
            (eng or nc.sync).dma_start(out=wqk_sb[:], in_=wqk_d[h][:])
            head_state[h] = dict(
                wqk=wqk_sb,
                bq=bias_sb[:, h : h + 1],
                bk=bias_sb[:, H + h : H + h + 1],
                b1e=bias_sb[:, 2 * H + h : 2 * H + h + 1],
                gam=bias_sb[:, 3 * H + h : 3 * H + h + 1],
                xres_lo=xl_lo[:, h, :],
                xres_hi=xl_hi[:, h, :],
            )

        # sync (SP) queue: xl first half rides the rings nearly alone so
        # the first real matmul group (xW1 chunk 0) unblocks earliest
        nc.sync.dma_start(out=xl_lo[:], in_=x16_d[:, :, 0:512])
        nc.sync.dma_start(out=w1t_sb[:], in_=w1t_d[:])
        emit_head_dmas(0)
        emit_head_dmas(1)
        nc.sync.dma_start(out=w1wv0_sb[:], in_=w1wv_d[:, :, 0:512])
        nc.sync.dma_start(out=w1wv1_sb[:], in_=w1wv_d[:, :, 512:1024])

        # scalar (ACT) queue: scalars + second x half + fp8 x
        nc.scalar.dma_start(out=bias_sb, in_=bias_d[:])
        nc.scalar.dma_start(out=xl_hi[:], in_=x16_d[:, :, 512:1024])
        nc.scalar.dma_start(out=w2o_sb, in_=w2o_d[:])
        nc.scalar.dma_start(out=x8_sb[:, 0:4, :], in_=x8_d[:, 0:4, :])
        nc.scalar.dma_start(out=x8_sb[:, 4:8, :], in_=x8_d[:, 4:8, :])

        w2t_sb = w2o_sb[:, 0:P]
        ones_sb = w2o_sb[:, P : 2 * P]
        b2_sb = bias_sb[:, 4 * H : 4 * H + 1]
        xw1_sb = consts.tile([P, T], f32, name="xw1_sb")
        vw1t_sb = consts.tile([P, ST, H * P], mdt, name="vw1t_sb")

        # ---------------- warm-up: dummy matmuls on the memset tile release
        # the HAM clock gate (~3.4us of PE busy) while input DMAs stream in.
        ps_w = psB.tile([P, TCW], f32, name="ps_w", tag="zf")
        for i in range(NWARM):
            mm(ps_w, dumm_sb[:, 0:P], dumm_sb[:, P : P + TCW], True, True)

        chunk_state = {}

        def emit_head_qk(h):
            hs = head_state[h]
            wqk_sb = hs["wqk"]
            q_sb = qkp.tile([P, T], ldt, name="q_sb", tag="q")
            k_sb = qkp.tile([P, T], ldt, name="k_sb", tag="k")
            out_sb = outp.tile([P, T], ldt, name="out_sb", tag="ob")
            # K then Q per t-half (first two groups only need the first x
            # half); bias-adds hide under later MM groups
            for t2 in range(2):
                tsl = slice(t2 * 512, (t2 + 1) * 512)
                xh = xl_lo if t2 == 0 else xl_hi
                ps_k = psB.tile([P, TCW], f32, name="ps_k", tag="qk", bufs=2)
                for ci in range(CT):
                    mm(ps_k, wqk_sb[:, ci, P : 2 * P], xh[:, ci, :], ci == 0, ci == CT - 1)
                nc.vector.tensor_scalar_add(out=k_sb[:, tsl], in0=ps_k, scalar1=hs["bk"])
                ps_q = psB.tile([P, TCW], f32, name="ps_q", tag="qk", bufs=2)
                for ci in range(CT):
                    mm(ps_q, wqk_sb[:, ci, 0:P], xh[:, ci, :], ci == 0, ci == CT - 1)
                nc.vector.tensor_scalar_add(out=q_sb[:, tsl], in0=ps_q, scalar1=hs["bq"])
            hs["q"] = q_sb
            hs["k"] = k_sb
            hs["out"] = out_sb

        def emit_s1_half(c, first):
            h, t_off, t_w = c
            hs = head_state[h]
            tsl = slice(t_off, t_off + t_w)
            if first:
                et_sb = expp.tile([P, ST, TCW], mdt, name="et_sb", tag="exp")
                chunk_state[c] = dict(et=et_sb)
            else:
                et_sb = chunk_state[c]["et"]
            rng = range(0, ST // 2) if first else range(ST // 2, ST)
            for si in rng:
                ps_e = psA.tile([P, TCW], f32, name="ps_e", tag="acc")
                mm(ps_e[:, :t_w], hs["k"][:, si * P : (si + 1) * P], hs["q"][:, tsl], True, True)
                nc.scalar.activation(out=et_sb[:, si, :t_w], in_=ps_e[:, :t_w], func=AF.Exp)

        def emit_s2_mm(c, late=False):
            h, t_off, t_w = c
            cs = chunk_state[c]
            et_sb = cs["et"]
            # late chunks borrow the dead QK-conv psum banks: the single
            # 'oo' bank would serialize each drain S2 behind the previous
            # chunk's DVE chain
            if late:
                ps_o = psB.tile([P, TCW], f32, name="ps_o", tag="qk", bufs=2)
            else:
                ps_o = psB.tile([P, TCW], f32, name="ps_o", tag="oo")
            for si in range(ST):
                mm(
                    ps_o[:, :t_w],
                    vw1t_sb[:, si, h * P : (h + 1) * P],
                    et_sb[:, si, :t_w],
                    si == 0,
                    si == ST - 1,
                )
            cs["ps_o"] = ps_o

        def emit_s2_s3(c, late=False):
            h, t_off, t_w = c
            hs = head_state[h]
            cs = chunk_state[c]
            tsl = slice(t_off, t_off + t_w)
            et_sb = cs["et"]
            ps_o = cs["ps_o"]
            if late:
                # drain chunks: Z as 8 accumulating ones-matmuls straight off
                # the et tiles -- PE is idle here while the serial DVE chain
                # is the drain bottleneck, so skip the DVE tree entirely
                ps_z = psB.tile([P, TCW], f32, name="ps_z", tag="oo")
                for si in range(ST):
                    mm(ps_z[:, :t_w], ones_sb, et_sb[:, si, :t_w], si == 0, si == ST - 1)
            else:
                # Z: tree-sum the 8 s-tiles on DVE (free-dim adds), then one
                # ones-matmul for the partition reduction + broadcast.
                r1 = hbuf.tile([P, 4, TCW], mdt, name="r1", tag="r1")
                nc.vector.tensor_add(r1[:, :, :t_w], et_sb[:, 0:4, :t_w], et_sb[:, 4:8, :t_w])
                r2 = hbuf.tile([P, 2, TCW], mdt, name="r2", tag="r2")
                nc.vector.tensor_add(r2[:, :, :t_w], r1[:, 0:2, :t_w], r1[:, 2:4, :t_w])
                etsum = hbuf.tile([P, TCW], mdt, name="etsum", tag="etsum")
                nc.vector.tensor_add(etsum[:, :t_w], r2[:, 0, :t_w], r2[:, 1, :t_w])
                ps_z = psB.tile([P, TCW], f32, name="ps_z", tag="zf")
                mm(ps_z[:, :t_w], ones_sb, etsum[:, :t_w], True, True)
            # fc1 = relu(gamma * oW1/Z + xW1 + b1eff): DVE chain, relu on ACT
            izg = hbuf.tile([P, TCW], f32, name="izg", tag="izg")
            nc.vector.reciprocal_approx_fast(out=izg[:, :t_w], in_=ps_z[:, :t_w])
            t1 = hbuf.tile([P, TCW], f32, name="t1", tag="t1")
            nc.vector.scalar_tensor_tensor(
                out=t1[:, :t_w], in0=ps_o[:, :t_w], scalar=hs["gam"], in1=izg[:, :t_w],
                op0=ALU.mult, op1=ALU.mult,
            )
            t2t = hbuf.tile([P, TCW], f32, name="t2t", tag="t2t")
            nc.vector.scalar_tensor_tensor(
                out=t2t[:, :t_w], in0=t1[:, :t_w], scalar=hs["b1e"], in1=xw1_sb[:, tsl],
                op0=ALU.add, op1=ALU.add,
            )
            fc1 = hbuf.tile([P, TCW], mdt, name="fc1", tag="fc1")
            nc.scalar.activation(out=fc1[:, :t_w], in_=t2t[:, :t_w], func=AF.Relu)
            cs["fc1"] = fc1

        def emit_s4_s5(c, dve_add=False, dma_eng=None, late=False):
            h, t_off, t_w = c
            hs = head_state[h]
            cs = chunk_state[c]
            tsl = slice(t_off, t_off + t_w)
            if late:
                ps_f = psA.tile([P, TCW], f32, name="ps_f", tag="acc")
            else:
                ps_f = psB.tile([P, TCW], f32, name="ps_f", tag="zf")
            mm(ps_f[:, :t_w], w2t_sb, cs["fc1"][:, :t_w], True, True)
            ot = hbuf.tile([P, TCW], f32, name="ot", tag="ot")
            nc.scalar.activation(out=ot[:, :t_w], in_=ps_f[:, :t_w], func=AF.Relu, bias=b2_sb)
            adder = nc.vector if dve_add else nc.gpsimd
            if t_off < 512:
                xres = hs["xres_lo"][:, t_off : t_off + t_w]
            else:
                xres = hs["xres_hi"][:, t_off - 512 : t_off - 512 + t_w]
            adder.tensor_add(hs["out"][:, tsl], ot[:, :t_w], xres)
            out_all = out_d[:]
            (dma_eng or nc.sync).dma_start(
                out=bass.AP(
                    tensor=out_all.tensor,
                    offset=h * T + t_off,
                    ap=[[H * T, P], [1, t_w]],
                ),
                in_=hs["out"][:, tsl],
            )

        # ---- phase A: xW1 chunk 0 (needs only w1t + first x half), head-0
        # QK (first two groups need only the first x half), xW1 chunk 1,
        # head-1 QK, then vW1T once the fp8 pair has streamed in.
        def emit_xw1(t2):
            tsl = slice(t2 * 512, (t2 + 1) * 512)
            xh = xl_lo if t2 == 0 else xl_hi
            ps_x = psA.tile([P, TCW], f32, name="ps_x", tag="acc")
            for ci in range(CT):
                mm(ps_x, w1t_sb[:, ci, :], xh[:, ci, :], ci == 0, ci == CT - 1)
            nc.scalar.activation(out=xw1_sb[:, tsl], in_=ps_x, func=AF.Copy)

        chunks = [(h, t2 * 512, 512) for h in range(H - 1) for t2 in range(2)]
        # taper the final chunks: chain latency scales with width
        chunks += [(7, 0, 512), (7, 512, 256), (7, 768, 128), (7, 896, 128)]
        N = len(chunks)

        emit_xw1(0)
        emit_head_qk(0)
        emit_xw1(1)
        emit_head_qk(1)

        # vW1T = x.T @ (W1 Wv).T for all heads, fp8 DoubleRow: 2 channel
        # tiles contract per matmul ([P, 2, .] pair slices on both operands).
        # jh-major so the first w1wv half alone unblocks the first 8 groups.
        for jh in range(2):
            jsl = slice(jh * 512, (jh + 1) * 512)
            wv = w1wv0_sb if jh == 0 else w1wv1_sb
            for si in range(ST):
                ps_v = psA.tile([P, TCW], f32, name="ps_v", tag="acc")
                for a in range(CT // 2):
                    mm(
                        ps_v,
                        x8_sb[:, 2 * a : 2 * a + 2, si * P : (si + 1) * P],
                        wv[:, 2 * a : 2 * a + 2, :],
                        a == 0,
                        a == CT // 2 - 1,
                        perf_mode=DR,
                    )
                nc.scalar.activation(out=vw1t_sb[:, si, jsl], in_=ps_v, func=AF.Copy)

        # Steady-state iteration (S4 BEFORE S3: FC2(c-3)'s psum bank was
        # freed by recip(c-3) a full period ago, while Z(c-2) -- emitted
        # last -- keeps the in-order PE from stalling on the DVE chain).
        # Tail: S1 of the last two (128-wide) chunks is pulled one
        # iteration early (expp bufs=6 holds them) so their exp chains
        # complete before the drain's S2/S3/S4 cascade needs them.
        for i, c in enumerate(chunks):
            h, t_off, _ = c
            if t_off == 0:
                if h <= 5:
                    emit_head_dmas(h + 2)
                if 1 <= h <= 6:
                    emit_head_qk(h + 1)
            if i <= N - 3:
                emit_s1_half(c, True)
            if i >= 2:
                emit_s2_mm(chunks[i - 2], late=(i - 2 >= N - 5))
            if i <= N - 3:
                emit_s1_half(c, False)
            if i >= 3:
                emit_s4_s5(chunks[i - 3], late=(i - 3 >= N - 5))
            if i >= 2:
                emit_s2_s3(chunks[i - 2], late=(i - 2 >= N - 5))
            if i == N - 3:
                emit_s1_half(chunks[N - 2], True)
                emit_s1_half(chunks[N - 2], False)
            if i == N - 2:
                emit_s1_half(chunks[N - 1], True)
                emit_s1_half(chunks[N - 1], False)
        emit_s2_mm(chunks[N - 2], late=True)
        emit_s4_s5(chunks[N - 3], late=True)
        emit_s2_s3(chunks[N - 2], late=True)
        emit_s2_mm(chunks[N - 1], late=True)
        emit_s2_s3(chunks[N - 1], late=True)
        emit_s4_s5(chunks[N - 2], dve_add=True, dma_eng=nc.scalar, late=True)
        emit_s4_s5(chunks[N - 1], dve_add=True, dma_eng=nc.sync, late=True)

    nc.compile()
    return nc


def _prepare_inputs(inputs):
    import ml_dtypes

    f16 = np.float16
    bf16 = ml_dtypes.bfloat16
    f8 = ml_dtypes.float8_e4m3fn

    x = np.ascontiguousarray(np.asarray(inputs["x"], dtype=np.float32))
    Wq = np.asarray(inputs["Wq"], dtype=np.float32)
    bq = np.asarray(inputs["bq"], dtype=np.float32)
    Wk = np.asarray(inputs["Wk"], dtype=np.float32)
    bk = np.asarray(inputs["bk"], dtype=np.float32)
    Wv = np.asarray(inputs["Wv"], dtype=np.float32)
    bv = np.asarray(inputs["bv"], dtype=np.float32)
    gamma = np.asarray(inputs["gamma"], dtype=np.float32)
    W1 = np.asarray(inputs["W1"], dtype=np.float32)
    b1 = np.asarray(inputs["b1"], dtype=np.float32)
    W2 = np.asarray(inputs["W2"], dtype=np.float32)
    b2 = np.asarray(inputs["b2"], dtype=np.float32)

    # channel permutation c = 8p + ci: plain reshape of the (C, x) transposes
    # wqk[h, p, ci, 0:128] = Wq[h, :, 8p+ci]; 128:256 for Wk
    wqk = np.empty((H, P, CT, 256), dtype=np.float32)
    for h in range(H):
        wqk[h, :, :, 0:P] = Wq[h].T.reshape(P, CT, P)
        wqk[h, :, :, P : 2 * P] = Wk[h].T.reshape(P, CT, P)

    # w1wv[p, ci, h*128+j] = WVS * (W1 @ Wv[h]).T[8p+ci, j], fp8 with a x16
    # range pre-scale; compensated by shipping gamma/WVS
    w1wvT = np.concatenate([(W1 @ Wv[h]).T for h in range(H)], axis=1)  # (C, H*128)
    w1wv = w1wvT.reshape(P, CT, H * P) * WVS

    w1t = W1.T.reshape(P, CT, P)
    w2o = np.concatenate([W2.T, np.ones((P, P), dtype=np.float32)], axis=1)

    b1v = bv @ W1.T  # (H, P): b1v[h] = W1 @ bv[h]
    b1e = b1[None, :] + gamma[:, None] * b1v  # (H, P)
    # bias[:, h]=bq[h]; [:, H+h]=bk[h]; [:, 2H+h]=b1e[h]; [:, 3H+h]=gam/WVS; [:, 4H]=b2
    bias = np.empty((P, 4 * H + 1), dtype=np.float32)
    bias[:, 0:H] = bq.T
    bias[:, H : 2 * H] = bk.T
    bias[:, 2 * H : 3 * H] = b1e.T
    bias[:, 3 * H : 4 * H] = np.tile(gamma[None, :] / WVS, (P, 1))
    bias[:, 4 * H] = b2

    shared = {
        "wqk": np.ascontiguousarray(wqk.astype(f16)),
        "w1wv": np.ascontiguousarray(w1wv.astype(f8)),
        "w1t": np.ascontiguousarray(w1t.astype(f16)),
        "w2o": np.ascontiguousarray(w2o.astype(bf16)),
        "bias": bias,
    }
    in_maps = []
    for b in range(B):
        m = dict(shared)
        xr = x[b].reshape(P, CT, T)
        m["x16"] = np.ascontiguousarray(xr.astype(f16))
        m["x8"] = np.ascontiguousarray(xr.astype(f8))
        in_maps.append(m)
    return in_maps


def kernel(**inputs):
    from concourse.bass_utils import run_bass_kernel_spmd

    if "nc" not in _module_cache:
        _module_cache["nc"] = _build_module()
    nc = _module_cache["nc"]

    in_maps = _prepare_inputs(inputs)
    res = run_bass_kernel_spmd(nc, in_maps, core_ids=list(range(B)))
    out = np.stack([res.results[b]["out"] for b in range(B)], axis=0)
    return out.astype(np.float32)



# revision 12
# speedup vs baseline: 1.0721x; 1.0073x over previous
"""Trainium2 Bass kernel for nn_MultiHeadAttention_9491877724818.

Math (per batch b, head h), reformulated from the reference:
    q = Wq_h @ x_b + bq          (128, T)
    k = Wk_h @ x_b + bk          (128, T)
    eT[s,t] = (k.T @ q)[s,t]     == energy[t,s]; softmax over s (partition dim)
    expET = exp(eT)              (no max subtraction: |logit| <= ~70, fp32-safe)
    Z[t] = sum_s expET[s,t]      (PE ones-matmul -> broadcast across partitions)
Key algebraic folding: heads only enter the output through W1 (DFC1=128 rows),
so the huge Wv (C x C) conv and o = v @ attn (each 2.1 GF/bh) collapse into
128-channel products:
    vW1T[s,j]  = (x_b.T @ (W1 @ Wv_h).T)[s,j]          (T, 128)
    oW1raw[j,t]= sum_s vW1T[s,j] expET[s,t]            (128, T)
    fc1[j,t]   = relu(gamma_h * oW1raw[j,t]/Z[t] + xW1[b][j,t] + b1eff_h[j])
        where xW1 = W1 @ x_b, b1eff = b1 + gamma_h * (W1 @ bv_h)
        (softmax rows sum to 1 => v-bias passes through as a constant)
    out2[d,t]  = relu(W2 @ fc1 + b2)
    final[b, 8*d + h, t] = out2[d,t] + x[b, 8*d + h, t]

Sharding: data parallel - core i computes batch b=i entirely (all 8 heads).

Channel permutation: contraction over C is order-independent, so x lives in
SBUF as x'[p, ci] = x[8p + ci] (weights retiled to match). Then the residual
rows for head h (channels h, h+8, ..., h+8*127) are exactly x'[:, h, :] - a
free view of the f16 matmul copy of x, so the f32 x is never loaded at all.

DMA model (measured): every dma_start costs ~0.6us of sequencer issue time on
its queue (DIRECT2D, serialized per engine; only SP=nc.sync and ACT=nc.scalar
have HWDGE), and one dma_start's transfer runs on ~one ring (~60 GB/s), so
parallelism needs multiple descriptors. Hence: x as 16 per-(ci,half)
descriptors split across sync+scalar, per-head scalars packed into ONE (P,33)
descriptor, wqk split k/q per head with a 2-head lead, outputs on sync.

Dtypes: the logit path (QK convs + k.T@q) -> float16 (exp() amplifies absolute
logit error). Post-softmax path -> bfloat16 (exp outputs reach ~e^70). vW1T
(x.T @ (W1 Wv).T) runs in fp8e4m3 with DoubleRow (2 channel-tiles per matmul):
w1wv is pre-scaled x16 into fp8 range, compensated via gamma/16; quantization
noise lands behind gamma (~0.1) and two FC layers -> ~5e-3 of output absmax.
PSUM accumulation is fp32 throughout.

Software pipeline (PE executes its queue in order; ACT/DVE are strict FIFO):
    step i emits:  S1(c_i) eT+exp | S2(c_{i-2}) oW1+Z | S3(c_{i-2}) normalize
                   | S4+S5(c_{i-3}) FC2+store
The 2-step S1->S2 lag means every chunk's exps have ~2 chunk-periods of ACT
headroom, so the PE never waits on the exp chain - including at the drain,
where the final chunks also taper (512,256,128,128) to shrink chain latency.
Warm-up dummy matmuls on a memset tile cover the initial input-DMA window and
release the HAM clock gate before real work arrives. Residual adds run on the
otherwise idle GpSimd engine (a f16-operand ADD is a DVE slow path); the
final two chunks' run on DVE, with stores split across both DGE queues.
"""

import numpy as np

B, C, T, H, P = 8, 1024, 1024, 8, 128
CT = C // P      # 8 contraction k-tiles over channels
ST = T // P      # 8 s-tiles (softmax/partition dim)
TCW = 512        # t-chunk width (matmul moving free dim)
NWARM = 6       # warm-up dummy matmuls covering the startup DMA window
WVS = 16.0       # host pre-scale on w1wv for fp8 range

_module_cache = {}


def _build_module():
    from contextlib import ExitStack

    import concourse.bacc as bacc
    import concourse.bass as bass
    import concourse.mybir as mybir
    import concourse.tile as tile

    f32 = mybir.dt.float32
    ldt = mybir.dt.float16
    mdt = mybir.dt.bfloat16
    f8 = mybir.dt.float8e4
    AF = mybir.ActivationFunctionType
    ALU = mybir.AluOpType
    DR = mybir.MatmulPerfMode.DoubleRow

    nc = bacc.Bacc(trn_type="TRN2", name="mha_dp")

    # x16lo[p, ci, t] = x[8p + ci, t] for t<512, hi for t>=512: separate
    # contiguous tensors keep DMA descriptor counts at 128 x 4KB per start
    # (descriptor GENERATION at ~7ns/desc per HWDGE queue is the startup
    # bottleneck, not ring bandwidth)
    x16lo_d = nc.dram_tensor("x16lo", (P, CT, 512), ldt, kind="ExternalInput")
    x16hi_d = nc.dram_tensor("x16hi", (P, CT, 512), ldt, kind="ExternalInput")
    x8_d = nc.dram_tensor("x8", (P, CT, T), f8, kind="ExternalInput")
    # wqk[h, p, ci, 0:128]=Wq[h,:,8p+ci], 128:256 for Wk
    wqk_d = nc.dram_tensor("wqk", (H, P, CT, 256), ldt, kind="ExternalInput")
    w1wv0_d = nc.dram_tensor("w1wv0", (P, CT, 512), f8, kind="ExternalInput")
    w1wv1_d = nc.dram_tensor("w1wv1", (P, CT, 512), f8, kind="ExternalInput")
    w1t_d = nc.dram_tensor("w1t", (P, CT, P), ldt, kind="ExternalInput")
    # w2o[:, 0:128] = W2.T, w2o[:, 128:256] = ones
    w2o_d = nc.dram_tensor("w2o", (P, 2 * P), mdt, kind="ExternalInput")
    # bias[:, h]=bq[h]; [:, H+h]=bk[h]; [:, 2H+h]=b1e[h]; [:, 3H+h]=gamma[h]/WVS; [:, 4H]=b2
    bias_d = nc.dram_tensor("bias", (P, 4 * H + 1), f32, kind="ExternalInput")
    out_d = nc.dram_tensor("out", (C, T), ldt, kind="ExternalOutput")

    def mm(ps, lhsT, rhs, start, stop, **kw):
        nc.tensor.matmul(ps, lhsT, rhs, start=start, stop=stop, **kw)

    with tile.TileContext(nc) as tc, ExitStack() as ctx:
        consts = ctx.enter_context(tc.tile_pool(name="consts", bufs=1))
        psA = ctx.enter_context(tc.tile_pool(name="psA", bufs=4, space="PSUM"))
        psB = ctx.enter_context(tc.tile_pool(name="psB", bufs=1, space="PSUM"))

        wqkp = ctx.enter_context(tc.tile_pool(name="wqkp", bufs=3))
        qkp = ctx.enter_context(tc.tile_pool(name="qkp", bufs=3))
        expp = ctx.enter_context(tc.tile_pool(name="expp", bufs=6))
        hbuf = ctx.enter_context(tc.tile_pool(name="hbuf", bufs=2))
        outp = ctx.enter_context(tc.tile_pool(name="outp", bufs=3))

        # ---------------- startup DMAs: few LARGE dma_starts (each costs
        # ~0.6us of sequencer issue), split across the two HWDGE queues so
        # the first-needed bytes (w1t + x first halves) land by ~4.5us.
        # Descriptors of all dma_starts spread over all 16 rings (~360GB/s).
        dumm_sb = consts.tile([P, 128 + TCW], ldt, name="dumm_sb")
        nc.vector.memset(dumm_sb[:], 0.25)

        w1t_sb = consts.tile([P, CT, P], ldt, name="w1t_sb")
        # xl/w1wv split into half tiles: consumers of one half must not
        # inherit DMA dependencies on the other (tile-granular hazards)
        xl_lo = consts.tile([P, CT, 512], ldt, name="xl_lo")
        xl_hi = consts.tile([P, CT, 512], ldt, name="xl_hi")
        w1wv0_sb = consts.tile([P, CT, 512], f8, name="w1wv0_sb")
        w1wv1_sb = consts.tile([P, CT, 512], f8, name="w1wv1_sb")
        x8_sb = consts.tile([P, CT, T], f8, name="x8_sb")
        bias_sb = consts.tile([P, 4 * H + 1], f32, name="bias_sb")
        w2o_sb = consts.tile([P, 2 * P], mdt, name="w2o_sb")

        head_state = {}

        def emit_head_dmas(h, eng=None):
            wqk_sb = wqkp.tile([P, CT, 256], ldt, name="wqk_sb", tag="wqk")
            (eng or nc.sync).dma_start(out=wqk_sb[:], in_=wqk_d[h][:])
            head_state[h] = dict(
                wqk=wqk_sb,
                bq=bias_sb[:, h : h + 1],
                bk=bias_sb[:, H + h : H + h + 1],
                b1e=bias_sb[:, 2 * H + h : 2 * H + h + 1],
                gam=bias_sb[:, 3 * H + h : 3 * H + h + 1],
                xres_lo=xl_lo[:, h, :],
                xres_hi=xl_hi[:, h, :],
            )

        # xl first half split across BOTH queues (parallel desc-gen) so
        # the first real matmul group (xW1 chunk 0) unblocks earliest
        nc.sync.dma_start(out=xl_lo[:, 0:4, :], in_=x16lo_d[:, 0:4, :])
        nc.sync.dma_start(out=w1t_sb[:], in_=w1t_d[:])
        emit_head_dmas(0)
        emit_head_dmas(1)
        nc.sync.dma_start(out=w1wv0_sb[:], in_=w1wv0_d[:])
        nc.sync.dma_start(out=w1wv1_sb[:], in_=w1wv1_d[:])

        nc.scalar.dma_start(out=xl_lo[:, 4:8, :], in_=x16lo_d[:, 4:8, :])
        nc.scalar.dma_start(out=bias_sb, in_=bias_d[:])
        nc.scalar.dma_start(out=xl_hi[:, 0:4, :], in_=x16hi_d[:, 0:4, :])
        nc.scalar.dma_start(out=xl_hi[:, 4:8, :], in_=x16hi_d[:, 4:8, :])
        nc.scalar.dma_start(out=w2o_sb, in_=w2o_d[:])
        nc.scalar.dma_start(out=x8_sb[:, 0:4, :], in_=x8_d[:, 0:4, :])
        nc.scalar.dma_start(out=x8_sb[:, 4:8, :], in_=x8_d[:, 4:8, :])

        w2t_sb = w2o_sb[:, 0:P]
        ones_sb = w2o_sb[:, P : 2 * P]
        b2_sb = bias_sb[:, 4 * H : 4 * H + 1]
        xw1_sb = consts.tile([P, T], f32, name="xw1_sb")
        vw1t_sb = consts.tile([P, ST, H * P], mdt, name="vw1t_sb")

        # ---------------- warm-up: dummy matmuls on the memset tile release
        # the HAM clock gate (~3.4us of PE busy) while input DMAs stream in.
        ps_w = psB.tile([P, TCW], f32, name="ps_w", tag="zf")
        for i in range(NWARM):
            mm(ps_w, dumm_sb[:, 0:P], dumm_sb[:, P : P + TCW], True, True)

        chunk_state = {}

        def emit_head_qk(h):
            hs = head_state[h]
            wqk_sb = hs["wqk"]
            q_sb = qkp.tile([P, T], ldt, name="q_sb", tag="q")
            k_sb = qkp.tile([P, T], ldt, name="k_sb", tag="k")
            out_sb = outp.tile([P, T], ldt, name="out_sb", tag="ob")
            # K then Q per t-half (first two groups only need the first x
            # half); bias-adds hide under later MM groups
            for t2 in range(2):
                tsl = slice(t2 * 512, (t2 + 1) * 512)
                xh = xl_lo if t2 == 0 else xl_hi
                ps_k = psB.tile([P, TCW], f32, name="ps_k", tag="qk", bufs=2)
                for ci in range(CT):
                    mm(ps_k, wqk_sb[:, ci, P : 2 * P], xh[:, ci, :], ci == 0, ci == CT - 1)
                nc.vector.tensor_scalar_add(out=k_sb[:, tsl], in0=ps_k, scalar1=hs["bk"])
                ps_q = psB.tile([P, TCW], f32, name="ps_q", tag="qk", bufs=2)
                for ci in range(CT):
                    mm(ps_q, wqk_sb[:, ci, 0:P], xh[:, ci, :], ci == 0, ci == CT - 1)
                nc.vector.tensor_scalar_add(out=q_sb[:, tsl], in0=ps_q, scalar1=hs["bq"])
            hs["q"] = q_sb
            hs["k"] = k_sb
            hs["out"] = out_sb

        def emit_s1_half(c, first):
            h, t_off, t_w = c
            hs = head_state[h]
            tsl = slice(t_off, t_off + t_w)
            if first:
                et_sb = expp.tile([P, ST, TCW], mdt, name="et_sb", tag="exp")
                chunk_state[c] = dict(et=et_sb)
            else:
                et_sb = chunk_state[c]["et"]
            rng = range(0, ST // 2) if first else range(ST // 2, ST)
            for si in rng:
                ps_e = psA.tile([P, TCW], f32, name="ps_e", tag="acc")
                mm(ps_e[:, :t_w], hs["k"][:, si * P : (si + 1) * P], hs["q"][:, tsl], True, True)
                nc.scalar.activation(out=et_sb[:, si, :t_w], in_=ps_e[:, :t_w], func=AF.Exp)

        def emit_s2_mm(c, late=False):
            h, t_off, t_w = c
            cs = chunk_state[c]
            et_sb = cs["et"]
            # late chunks borrow the dead QK-conv psum banks: the single
            # 'oo' bank would serialize each drain S2 behind the previous
            # chunk's DVE chain
            if late:
                ps_o = psB.tile([P, TCW], f32, name="ps_o", tag="qk", bufs=2)
            else:
                ps_o = psB.tile([P, TCW], f32, name="ps_o", tag="oo")
            for si in range(ST):
                mm(
                    ps_o[:, :t_w],
                    vw1t_sb[:, si, h * P : (h + 1) * P],
                    et_sb[:, si, :t_w],
                    si == 0,
                    si == ST - 1,
                )
            cs["ps_o"] = ps_o

        def emit_s2_s3(c, late=False):
            h, t_off, t_w = c
            hs = head_state[h]
            cs = chunk_state[c]
            tsl = slice(t_off, t_off + t_w)
            et_sb = cs["et"]
            ps_o = cs["ps_o"]
            if late:
                # drain chunks: Z as 8 accumulating ones-matmuls straight off
                # the et tiles -- PE is idle here while the serial DVE chain
                # is the drain bottleneck, so skip the DVE tree entirely
                ps_z = psB.tile([P, TCW], f32, name="ps_z", tag="oo")
                for si in range(ST):
                    mm(ps_z[:, :t_w], ones_sb, et_sb[:, si, :t_w], si == 0, si == ST - 1)
            else:
                # Z: tree-sum the 8 s-tiles on DVE (free-dim adds), then one
                # ones-matmul for the partition reduction + broadcast.
                r1 = hbuf.tile([P, 4, TCW], mdt, name="r1", tag="r1")
                nc.vector.tensor_add(r1[:, :, :t_w], et_sb[:, 0:4, :t_w], et_sb[:, 4:8, :t_w])
                r2 = hbuf.tile([P, 2, TCW], mdt, name="r2", tag="r2")
                nc.vector.tensor_add(r2[:, :, :t_w], r1[:, 0:2, :t_w], r1[:, 2:4, :t_w])
                etsum = hbuf.tile([P, TCW], mdt, name="etsum", tag="etsum")
                nc.vector.tensor_add(etsum[:, :t_w], r2[:, 0, :t_w], r2[:, 1, :t_w])
                ps_z = psB.tile([P, TCW], f32, name="ps_z", tag="zf")
                mm(ps_z[:, :t_w], ones_sb, etsum[:, :t_w], True, True)
            # fc1 = relu(gamma * oW1/Z + xW1 + b1eff): DVE chain, relu on ACT
            izg = hbuf.tile([P, TCW], f32, name="izg", tag="izg")
            nc.vector.reciprocal_approx_fast(out=izg[:, :t_w], in_=ps_z[:, :t_w])
            t1 = hbuf.tile([P, TCW], f32, name="t1", tag="t1")
            nc.vector.scalar_tensor_tensor(
                out=t1[:, :t_w], in0=ps_o[:, :t_w], scalar=hs["gam"], in1=izg[:, :t_w],
                op0=ALU.mult, op1=ALU.mult,
            )
            t2t = hbuf.tile([P, TCW], f32, name="t2t", tag="t2t")
            nc.vector.scalar_tensor_tensor(
                out=t2t[:, :t_w], in0=t1[:, :t_w], scalar=hs["b1e"], in1=xw1_sb[:, tsl],
                op0=ALU.add, op1=ALU.add,
            )
            fc1 = hbuf.tile([P, TCW], mdt, name="fc1", tag="fc1")
            nc.scalar.activation(out=fc1[:, :t_w], in_=t2t[:, :t_w], func=AF.Relu)
            cs["fc1"] = fc1

        def emit_s4_s5(c, dve_add=False, dma_eng=None, late=False):
            h, t_off, t_w = c
            hs = head_state[h]
            cs = chunk_state[c]
            tsl = slice(t_off, t_off + t_w)
            if late:
                ps_f = psA.tile([P, TCW], f32, name="ps_f", tag="acc")
            else:
                ps_f = psB.tile([P, TCW], f32, name="ps_f", tag="zf")
            mm(ps_f[:, :t_w], w2t_sb, cs["fc1"][:, :t_w], True, True)
            ot = hbuf.tile([P, TCW], f32, name="ot", tag="ot")
            nc.scalar.activation(out=ot[:, :t_w], in_=ps_f[:, :t_w], func=AF.Relu, bias=b2_sb)
            adder = nc.vector if dve_add else nc.gpsimd
            if t_off < 512:
                xres = hs["xres_lo"][:, t_off : t_off + t_w]
            else:
                xres = hs["xres_hi"][:, t_off - 512 : t_off - 512 + t_w]
            adder.tensor_add(hs["out"][:, tsl], ot[:, :t_w], xres)
            out_all = out_d[:]
            (dma_eng or nc.sync).dma_start(
                out=bass.AP(
                    tensor=out_all.tensor,
                    offset=h * T + t_off,
                    ap=[[H * T, P], [1, t_w]],
                ),
                in_=hs["out"][:, tsl],
            )

        # ---- phase A: xW1 chunk 0 (needs only w1t + first x half), head-0
        # QK (first two groups need only the first x half), xW1 chunk 1,
        # head-1 QK, then vW1T once the fp8 pair has streamed in.
        def emit_xw1(t2):
            tsl = slice(t2 * 512, (t2 + 1) * 512)
            xh = xl_lo if t2 == 0 else xl_hi
            ps_x = psA.tile([P, TCW], f32, name="ps_x", tag="acc")
            for ci in range(CT):
                mm(ps_x, w1t_sb[:, ci, :], xh[:, ci, :], ci == 0, ci == CT - 1)
            nc.scalar.activation(out=xw1_sb[:, tsl], in_=ps_x, func=AF.Copy)

        chunks = [(h, t2 * 512, 512) for h in range(H - 1) for t2 in range(2)]
        # taper the final chunks: chain latency scales with width
        chunks += [(7, 0, 512), (7, 512, 256), (7, 768, 128), (7, 896, 128)]
        N = len(chunks)

        emit_xw1(0)
        emit_head_qk(0)
        emit_xw1(1)
        emit_head_qk(1)

        # vW1T = x.T @ (W1 Wv).T for all heads, fp8 DoubleRow: 2 channel
        # tiles contract per matmul ([P, 2, .] pair slices on both operands).
        # jh-major so the first w1wv half alone unblocks the first 8 groups.
        for jh in range(2):
            jsl = slice(jh * 512, (jh + 1) * 512)
            wv = w1wv0_sb if jh == 0 else w1wv1_sb
            for si in range(ST):
                ps_v = psA.tile([P, TCW], f32, name="ps_v", tag="acc")
                for a in range(CT // 2):
                    mm(
                        ps_v,
                        x8_sb[:, 2 * a : 2 * a + 2, si * P : (si + 1) * P],
                        wv[:, 2 * a : 2 * a + 2, :],
                        a == 0,
                        a == CT // 2 - 1,
                        perf_mode=DR,
                    )
                nc.scalar.activation(out=vw1t_sb[:, si, jsl], in_=ps_v, func=AF.Copy)

        # Steady-state iteration (S4 BEFORE S3: FC2(c-3)'s psum bank was
        # freed by recip(c-3) a full period ago, while Z(c-2) -- emitted
        # last -- keeps the in-order PE from stalling on the DVE chain).
        # Tail: S1 of the last two (128-wide) chunks is pulled one
        # iteration early (expp bufs=6 holds them) so their exp chains
        # complete before the drain's S2/S3/S4 cascade needs them.
        for i, c in enumerate(chunks):
            h, t_off, _ = c
            if t_off == 0:
                if h <= 5:
                    emit_head_dmas(h + 2)
                if 1 <= h <= 6:
                    emit_head_qk(h + 1)
            if i <= N - 3:
                emit_s1_half(c, True)
            if i >= 2:
                emit_s2_mm(chunks[i - 2], late=(i - 2 >= N - 5))
            if i <= N - 3:
                emit_s1_half(c, False)
            if i >= 3:
                emit_s4_s5(chunks[i - 3], late=(i - 3 >= N - 5))
            if i >= 2:
                emit_s2_s3(chunks[i - 2], late=(i - 2 >= N - 5))
            if i == N - 3:
                emit_s1_half(chunks[N - 2], True)
                emit_s1_half(chunks[N - 2], False)
            if i == N - 2:
                emit_s1_half(chunks[N - 1], True)
                emit_s1_half(chunks[N - 1], False)
        emit_s2_mm(chunks[N - 2], late=True)
        emit_s4_s5(chunks[N - 3], late=True)
        emit_s2_s3(chunks[N - 2], late=True)
        emit_s2_mm(chunks[N - 1], late=True)
        emit_s2_s3(chunks[N - 1], late=True)
        emit_s4_s5(chunks[N - 2], dve_add=True, dma_eng=nc.scalar, late=True)
        emit_s4_s5(chunks[N - 1], dve_add=True, dma_eng=nc.sync, late=True)

    nc.compile()
    return nc


def _prepare_inputs(inputs):
    import ml_dtypes

    f16 = np.float16
    bf16 = ml_dtypes.bfloat16
    f8 = ml_dtypes.float8_e4m3fn

    x = np.ascontiguousarray(np.asarray(inputs["x"], dtype=np.float32))
    Wq = np.asarray(inputs["Wq"], dtype=np.float32)
    bq = np.asarray(inputs["bq"], dtype=np.float32)
    Wk = np.asarray(inputs["Wk"], dtype=np.float32)
    bk = np.asarray(inputs["bk"], dtype=np.float32)
    Wv = np.asarray(inputs["Wv"], dtype=np.float32)
    bv = np.asarray(inputs["bv"], dtype=np.float32)
    gamma = np.asarray(inputs["gamma"], dtype=np.float32)
    W1 = np.asarray(inputs["W1"], dtype=np.float32)
    b1 = np.asarray(inputs["b1"], dtype=np.float32)
    W2 = np.asarray(inputs["W2"], dtype=np.float32)
    b2 = np.asarray(inputs["b2"], dtype=np.float32)

    # channel permutation c = 8p + ci: plain reshape of the (C, x) transposes
    # wqk[h, p, ci, 0:128] = Wq[h, :, 8p+ci]; 128:256 for Wk
    wqk = np.empty((H, P, CT, 256), dtype=np.float32)
    for h in range(H):
        wqk[h, :, :, 0:P] = Wq[h].T.reshape(P, CT, P)
        wqk[h, :, :, P : 2 * P] = Wk[h].T.reshape(P, CT, P)

    # w1wv[p, ci, h*128+j] = WVS * (W1 @ Wv[h]).T[8p+ci, j], fp8 with a x16
    # range pre-scale; compensated by shipping gamma/WVS
    w1wvT = np.concatenate([(W1 @ Wv[h]).T for h in range(H)], axis=1)  # (C, H*128)
    w1wv = w1wvT.reshape(P, CT, H * P) * WVS
    w1wv8 = w1wv.astype(f8)

    w1t = W1.T.reshape(P, CT, P)
    w2o = np.concatenate([W2.T, np.ones((P, P), dtype=np.float32)], axis=1)

    b1v = bv @ W1.T  # (H, P): b1v[h] = W1 @ bv[h]
    b1e = b1[None, :] + gamma[:, None] * b1v  # (H, P)
    # bias[:, h]=bq[h]; [:, H+h]=bk[h]; [:, 2H+h]=b1e[h]; [:, 3H+h]=gam/WVS; [:, 4H]=b2
    bias = np.empty((P, 4 * H + 1), dtype=np.float32)
    bias[:, 0:H] = bq.T
    bias[:, H : 2 * H] = bk.T
    bias[:, 2 * H : 3 * H] = b1e.T
    bias[:, 3 * H : 4 * H] = np.tile(gamma[None, :] / WVS, (P, 1))
    bias[:, 4 * H] = b2

    shared = {
        "wqk": np.ascontiguousarray(wqk.astype(f16)),
        "w1wv0": np.ascontiguousarray(w1wv8[:, :, 0:512]),
        "w1wv1": np.ascontiguousarray(w1wv8[:, :, 512:1024]),
        "w1t": np.ascontiguousarray(w1t.astype(f16)),
        "w2o": np.ascontiguousarray(w2o.astype(bf16)),
        "bias": bias,
    }
    in_maps = []
    for b in range(B):
        m = dict(shared)
        xr = x[b].reshape(P, CT, T)
        x16 = xr.astype(f16)
        m["x16lo"] = np.ascontiguousarray(x16[:, :, 0:512])
        m["x16hi"] = np.ascontiguousarray(x16[:, :, 512:1024])
        m["x8"] = np.ascontiguousarray(xr.astype(f8))
        in_maps.append(m)
    return in_maps


def kernel(**inputs):
    from concourse.bass_utils import run_bass_kernel_spmd

    if "nc" not in _module_cache:
        _module_cache["nc"] = _build_module()
    nc = _module_cache["nc"]

    in_maps = _prepare_inputs(inputs)
    res = run_bass_kernel_spmd(nc, in_maps, core_ids=list(range(B)))
    out = np.stack([res.results[b]["out"] for b in range(B)], axis=0)
    return out.astype(np.float32)



# revision 13
# speedup vs baseline: 1.0794x; 1.0068x over previous
"""Trainium2 Bass kernel for nn_MultiHeadAttention_9491877724818.

Math (per batch b, head h), reformulated from the reference:
    q = Wq_h @ x_b + bq          (128, T)
    k = Wk_h @ x_b + bk          (128, T)
    eT[s,t] = (k.T @ q)[s,t]     == energy[t,s]; softmax over s (partition dim)
    expET = exp(eT)              (no max subtraction: |logit| <= ~70, fp32-safe)
    Z[t] = sum_s expET[s,t]      (PE ones-matmul -> broadcast across partitions)
Key algebraic folding: heads only enter the output through W1 (DFC1=128 rows),
so the huge Wv (C x C) conv and o = v @ attn (each 2.1 GF/bh) collapse into
128-channel products:
    vW1T[s,j]  = (x_b.T @ (W1 @ Wv_h).T)[s,j]          (T, 128)
    oW1raw[j,t]= sum_s vW1T[s,j] expET[s,t]            (128, T)
    fc1[j,t]   = relu(gamma_h * oW1raw[j,t]/Z[t] + xW1[b][j,t] + b1eff_h[j])
        where xW1 = W1 @ x_b, b1eff = b1 + gamma_h * (W1 @ bv_h)
        (softmax rows sum to 1 => v-bias passes through as a constant)
    out2[d,t]  = relu(W2 @ fc1 + b2)
    final[b, 8*d + h, t] = out2[d,t] + x[b, 8*d + h, t]

Sharding: data parallel - core i computes batch b=i entirely (all 8 heads).

Channel permutation: contraction over C is order-independent, so x lives in
SBUF as x'[p, ci] = x[8p + ci] (weights retiled to match). Then the residual
rows for head h (channels h, h+8, ..., h+8*127) are exactly x'[:, h, :] - a
free view of the f16 matmul copy of x, so the f32 x is never loaded at all.

DMA model (measured): every dma_start costs ~0.6us of sequencer issue time on
its queue (DIRECT2D, serialized per engine; only SP=nc.sync and ACT=nc.scalar
have HWDGE), and one dma_start's transfer runs on ~one ring (~60 GB/s), so
parallelism needs multiple descriptors. Hence: x as 16 per-(ci,half)
descriptors split across sync+scalar, per-head scalars packed into ONE (P,33)
descriptor, wqk split k/q per head with a 2-head lead, outputs on sync.

Dtypes: the logit path (QK convs + k.T@q) -> float16 (exp() amplifies absolute
logit error). Post-softmax path -> bfloat16 (exp outputs reach ~e^70). vW1T
(x.T @ (W1 Wv).T) runs in fp8e4m3 with DoubleRow (2 channel-tiles per matmul):
w1wv is pre-scaled x16 into fp8 range, compensated via gamma/16; quantization
noise lands behind gamma (~0.1) and two FC layers -> ~5e-3 of output absmax.
PSUM accumulation is fp32 throughout.

Software pipeline (PE executes its queue in order; ACT/DVE are strict FIFO):
    step i emits:  S1(c_i) eT+exp | S2(c_{i-2}) oW1+Z | S3(c_{i-2}) normalize
                   | S4+S5(c_{i-3}) FC2+store
The 2-step S1->S2 lag means every chunk's exps have ~2 chunk-periods of ACT
headroom, so the PE never waits on the exp chain - including at the drain,
where the final chunks also taper (512,256,128,128) to shrink chain latency.
Warm-up dummy matmuls on a memset tile cover the initial input-DMA window and
release the HAM clock gate before real work arrives. Residual adds run on the
otherwise idle GpSimd engine (a f16-operand ADD is a DVE slow path); the
final two chunks' run on DVE, with stores split across both DGE queues.
"""

import numpy as np

B, C, T, H, P = 8, 1024, 1024, 8, 128
CT = C // P      # 8 contraction k-tiles over channels
ST = T // P      # 8 s-tiles (softmax/partition dim)
TCW = 512        # t-chunk width (matmul moving free dim)
NWARM = 9       # warm-up dummy matmuls covering the startup DMA window
WVS = 16.0       # host pre-scale on w1wv for fp8 range

_module_cache = {}


def _build_module():
    from contextlib import ExitStack

    import concourse.bacc as bacc
    import concourse.bass as bass
    import concourse.mybir as mybir
    import concourse.tile as tile

    f32 = mybir.dt.float32
    ldt = mybir.dt.float16
    mdt = mybir.dt.bfloat16
    f8 = mybir.dt.float8e4
    AF = mybir.ActivationFunctionType
    ALU = mybir.AluOpType
    DR = mybir.MatmulPerfMode.DoubleRow

    nc = bacc.Bacc(trn_type="TRN2", name="mha_dp")

    # x16lo[p, ci, t] = x[8p + ci, t] for t<512, hi for t>=512: separate
    # contiguous tensors keep DMA descriptor counts at 128 x 4KB per start
    # (descriptor GENERATION at ~7ns/desc per HWDGE queue is the startup
    # bottleneck, not ring bandwidth)
    x16lo_d = nc.dram_tensor("x16lo", (P, CT, 512), ldt, kind="ExternalInput")
    x16hi_d = nc.dram_tensor("x16hi", (P, CT, 512), ldt, kind="ExternalInput")
    x8_d = nc.dram_tensor("x8", (P, CT, T), f8, kind="ExternalInput")
    # wqk[h, p, ci, 0:128]=Wq[h,:,8p+ci], 128:256 for Wk
    wqk_d = nc.dram_tensor("wqk", (H, P, CT, 256), ldt, kind="ExternalInput")
    w1wv0_d = nc.dram_tensor("w1wv0", (P, CT, 512), f8, kind="ExternalInput")
    w1wv1_d = nc.dram_tensor("w1wv1", (P, CT, 512), f8, kind="ExternalInput")
    w1t_d = nc.dram_tensor("w1t", (P, CT, P), ldt, kind="ExternalInput")
    # w2o[:, 0:128] = W2.T, w2o[:, 128:256] = ones
    w2o_d = nc.dram_tensor("w2o", (P, 2 * P), mdt, kind="ExternalInput")
    # bias[:, h]=bq[h]; [:, H+h]=bk[h]; [:, 2H+h]=b1e[h]; [:, 3H+h]=gamma[h]/WVS; [:, 4H]=b2
    bias_d = nc.dram_tensor("bias", (P, 4 * H + 1), f32, kind="ExternalInput")
    out_d = nc.dram_tensor("out", (C, T), ldt, kind="ExternalOutput")

    def mm(ps, lhsT, rhs, start, stop, **kw):
        nc.tensor.matmul(ps, lhsT, rhs, start=start, stop=stop, **kw)

    with tile.TileContext(nc) as tc, ExitStack() as ctx:
        consts = ctx.enter_context(tc.tile_pool(name="consts", bufs=1))
        psA = ctx.enter_context(tc.tile_pool(name="psA", bufs=4, space="PSUM"))
        psB = ctx.enter_context(tc.tile_pool(name="psB", bufs=1, space="PSUM"))

        wqkp = ctx.enter_context(tc.tile_pool(name="wqkp", bufs=3))
        qkp = ctx.enter_context(tc.tile_pool(name="qkp", bufs=3))
        expp = ctx.enter_context(tc.tile_pool(name="expp", bufs=6))
        hbuf = ctx.enter_context(tc.tile_pool(name="hbuf", bufs=2))
        outp = ctx.enter_context(tc.tile_pool(name="outp", bufs=3))

        # ---------------- startup DMAs: few LARGE dma_starts (each costs
        # ~0.6us of sequencer issue), split across the two HWDGE queues so
        # the first-needed bytes (w1t + x first halves) land by ~4.5us.
        # Descriptors of all dma_starts spread over all 16 rings (~360GB/s).
        dumm_sb = consts.tile([P, 128 + TCW], ldt, name="dumm_sb")
        nc.vector.memset(dumm_sb[:], 0.25)

        w1t_sb = consts.tile([P, CT, P], ldt, name="w1t_sb")
        # xl/w1wv split into half tiles: consumers of one half must not
        # inherit DMA dependencies on the other (tile-granular hazards)
        xl_lo = consts.tile([P, CT, 512], ldt, name="xl_lo")
        xl_hi = consts.tile([P, CT, 512], ldt, name="xl_hi")
        w1wv0_sb = consts.tile([P, CT, 512], f8, name="w1wv0_sb")
        w1wv1_sb = consts.tile([P, CT, 512], f8, name="w1wv1_sb")
        x8_sb = consts.tile([P, CT, T], f8, name="x8_sb")
        bias_sb = consts.tile([P, 4 * H + 1], f32, name="bias_sb")
        w2o_sb = consts.tile([P, 2 * P], mdt, name="w2o_sb")

        head_state = {}

        def emit_head_dmas(h, eng=None):
            wqk_sb = wqkp.tile([P, CT, 256], ldt, name="wqk_sb", tag="wqk")
            (eng or nc.sync).dma_start(out=wqk_sb[:], in_=wqk_d[h][:])
            head_state[h] = dict(
                wqk=wqk_sb,
                bq=bias_sb[:, h : h + 1],
                bk=bias_sb[:, H + h : H + h + 1],
                b1e=bias_sb[:, 2 * H + h : 2 * H + h + 1],
                gam=bias_sb[:, 3 * H + h : 3 * H + h + 1],
                xres_lo=xl_lo[:, h, :],
                xres_hi=xl_hi[:, h, :],
            )

        # DMA rings drain in enqueue order and the sync queue's transfers
        # start ~2us before the scalar queue's: xl_lo (the first real
        # matmul's input) goes first and alone on sync
        nc.sync.dma_start(out=xl_lo[:, 0:4, :], in_=x16lo_d[:, 0:4, :])
        nc.sync.dma_start(out=xl_lo[:, 4:8, :], in_=x16lo_d[:, 4:8, :])
        emit_head_dmas(0)
        emit_head_dmas(1)
        nc.sync.dma_start(out=w1wv0_sb[:], in_=w1wv0_d[:])
        nc.sync.dma_start(out=w1wv1_sb[:], in_=w1wv1_d[:])

        nc.scalar.dma_start(out=w1t_sb[:], in_=w1t_d[:])
        nc.scalar.dma_start(out=bias_sb, in_=bias_d[:])
        nc.scalar.dma_start(out=xl_hi[:, 0:4, :], in_=x16hi_d[:, 0:4, :])
        nc.scalar.dma_start(out=xl_hi[:, 4:8, :], in_=x16hi_d[:, 4:8, :])
        nc.scalar.dma_start(out=w2o_sb, in_=w2o_d[:])
        nc.scalar.dma_start(out=x8_sb[:, 0:4, :], in_=x8_d[:, 0:4, :])
        nc.scalar.dma_start(out=x8_sb[:, 4:8, :], in_=x8_d[:, 4:8, :])

        w2t_sb = w2o_sb[:, 0:P]
        ones_sb = w2o_sb[:, P : 2 * P]
        b2_sb = bias_sb[:, 4 * H : 4 * H + 1]
        xw1_sb = consts.tile([P, T], f32, name="xw1_sb")
        vw1t_sb = consts.tile([P, ST, H * P], mdt, name="vw1t_sb")

        # ---------------- warm-up: dummy matmuls on the memset tile release
        # the HAM clock gate (~3.4us of PE busy) while input DMAs stream in.
        ps_w = psB.tile([P, TCW], f32, name="ps_w", tag="zf")
        for i in range(NWARM):
            mm(ps_w, dumm_sb[:, 0:P], dumm_sb[:, P : P + TCW], True, True)

        chunk_state = {}

        def emit_head_qk(h):
            hs = head_state[h]
            wqk_sb = hs["wqk"]
            q_sb = qkp.tile([P, T], ldt, name="q_sb", tag="q")
            k_sb = qkp.tile([P, T], ldt, name="k_sb", tag="k")
            out_sb = outp.tile([P, T], ldt, name="out_sb", tag="ob")
            # K then Q per t-half (first two groups only need the first x
            # half); bias-adds hide under later MM groups
            for t2 in range(2):
                tsl = slice(t2 * 512, (t2 + 1) * 512)
                xh = xl_lo if t2 == 0 else xl_hi
                ps_k = psB.tile([P, TCW], f32, name="ps_k", tag="qk", bufs=2)
                for ci in range(CT):
                    mm(ps_k, wqk_sb[:, ci, P : 2 * P], xh[:, ci, :], ci == 0, ci == CT - 1)
                nc.vector.tensor_scalar_add(out=k_sb[:, tsl], in0=ps_k, scalar1=hs["bk"])
                ps_q = psB.tile([P, TCW], f32, name="ps_q", tag="qk", bufs=2)
                for ci in range(CT):
                    mm(ps_q, wqk_sb[:, ci, 0:P], xh[:, ci, :], ci == 0, ci == CT - 1)
                nc.vector.tensor_scalar_add(out=q_sb[:, tsl], in0=ps_q, scalar1=hs["bq"])
            hs["q"] = q_sb
            hs["k"] = k_sb
            hs["out"] = out_sb

        def emit_s1_half(c, first):
            h, t_off, t_w = c
            hs = head_state[h]
            tsl = slice(t_off, t_off + t_w)
            if first:
                et_sb = expp.tile([P, ST, TCW], mdt, name="et_sb", tag="exp")
                chunk_state[c] = dict(et=et_sb)
            else:
                et_sb = chunk_state[c]["et"]
            rng = range(0, ST // 2) if first else range(ST // 2, ST)
            for si in rng:
                ps_e = psA.tile([P, TCW], f32, name="ps_e", tag="acc")
                mm(ps_e[:, :t_w], hs["k"][:, si * P : (si + 1) * P], hs["q"][:, tsl], True, True)
                nc.scalar.activation(out=et_sb[:, si, :t_w], in_=ps_e[:, :t_w], func=AF.Exp)

        def emit_s2_mm(c, late=False):
            h, t_off, t_w = c
            cs = chunk_state[c]
            et_sb = cs["et"]
            # late chunks borrow the dead QK-conv psum banks: the single
            # 'oo' bank would serialize each drain S2 behind the previous
            # chunk's DVE chain
            if late:
                ps_o = psB.tile([P, TCW], f32, name="ps_o", tag="qk", bufs=2)
            else:
                ps_o = psB.tile([P, TCW], f32, name="ps_o", tag="oo")
            for si in range(ST):
                mm(
                    ps_o[:, :t_w],
                    vw1t_sb[:, si, h * P : (h + 1) * P],
                    et_sb[:, si, :t_w],
                    si == 0,
                    si == ST - 1,
                )
            cs["ps_o"] = ps_o

        def emit_s2_s3(c, late=False):
            h, t_off, t_w = c
            hs = head_state[h]
            cs = chunk_state[c]
            tsl = slice(t_off, t_off + t_w)
            et_sb = cs["et"]
            ps_o = cs["ps_o"]
            if late:
                # drain chunks: Z as 8 accumulating ones-matmuls straight off
                # the et tiles -- PE is idle here while the serial DVE chain
                # is the drain bottleneck, so skip the DVE tree entirely
                ps_z = psB.tile([P, TCW], f32, name="ps_z", tag="oo")
                for si in range(ST):
                    mm(ps_z[:, :t_w], ones_sb, et_sb[:, si, :t_w], si == 0, si == ST - 1)
            else:
                # Z: tree-sum the 8 s-tiles on DVE (free-dim adds), then one
                # ones-matmul for the partition reduction + broadcast.
                r1 = hbuf.tile([P, 4, TCW], mdt, name="r1", tag="r1")
                nc.vector.tensor_add(r1[:, :, :t_w], et_sb[:, 0:4, :t_w], et_sb[:, 4:8, :t_w])
                r2 = hbuf.tile([P, 2, TCW], mdt, name="r2", tag="r2")
                nc.vector.tensor_add(r2[:, :, :t_w], r1[:, 0:2, :t_w], r1[:, 2:4, :t_w])
                etsum = hbuf.tile([P, TCW], mdt, name="etsum", tag="etsum")
                nc.vector.tensor_add(etsum[:, :t_w], r2[:, 0, :t_w], r2[:, 1, :t_w])
                ps_z = psB.tile([P, TCW], f32, name="ps_z", tag="zf")
                mm(ps_z[:, :t_w], ones_sb, etsum[:, :t_w], True, True)
            # fc1 = relu(gamma * oW1/Z + xW1 + b1eff): DVE chain, relu on ACT
            izg = hbuf.tile([P, TCW], f32, name="izg", tag="izg")
            nc.vector.reciprocal_approx_fast(out=izg[:, :t_w], in_=ps_z[:, :t_w])
            t1 = hbuf.tile([P, TCW], f32, name="t1", tag="t1")
            nc.vector.scalar_tensor_tensor(
                out=t1[:, :t_w], in0=ps_o[:, :t_w], scalar=hs["gam"], in1=izg[:, :t_w],
                op0=ALU.mult, op1=ALU.mult,
            )
            t2t = hbuf.tile([P, TCW], f32, name="t2t", tag="t2t")
            nc.vector.scalar_tensor_tensor(
                out=t2t[:, :t_w], in0=t1[:, :t_w], scalar=hs["b1e"], in1=xw1_sb[:, tsl],
                op0=ALU.add, op1=ALU.add,
            )
            fc1 = hbuf.tile([P, TCW], mdt, name="fc1", tag="fc1")
            nc.scalar.activation(out=fc1[:, :t_w], in_=t2t[:, :t_w], func=AF.Relu)
            cs["fc1"] = fc1

        def emit_s4_s5(c, dve_add=False, dma_eng=None, late=False):
            h, t_off, t_w = c
            hs = head_state[h]
            cs = chunk_state[c]
            tsl = slice(t_off, t_off + t_w)
            if late:
                ps_f = psA.tile([P, TCW], f32, name="ps_f", tag="acc")
            else:
                ps_f = psB.tile([P, TCW], f32, name="ps_f", tag="zf")
            mm(ps_f[:, :t_w], w2t_sb, cs["fc1"][:, :t_w], True, True)
            ot = hbuf.tile([P, TCW], f32, name="ot", tag="ot")
            nc.scalar.activation(out=ot[:, :t_w], in_=ps_f[:, :t_w], func=AF.Relu, bias=b2_sb)
            adder = nc.vector if dve_add else nc.gpsimd
            if t_off < 512:
                xres = hs["xres_lo"][:, t_off : t_off + t_w]
            else:
                xres = hs["xres_hi"][:, t_off - 512 : t_off - 512 + t_w]
            adder.tensor_add(hs["out"][:, tsl], ot[:, :t_w], xres)
            out_all = out_d[:]
            (dma_eng or nc.sync).dma_start(
                out=bass.AP(
                    tensor=out_all.tensor,
                    offset=h * T + t_off,
                    ap=[[H * T, P], [1, t_w]],
                ),
                in_=hs["out"][:, tsl],
            )

        # ---- phase A: xW1 chunk 0 (needs only w1t + first x half), head-0
        # QK (first two groups need only the first x half), xW1 chunk 1,
        # head-1 QK, then vW1T once the fp8 pair has streamed in.
        def emit_xw1(t2):
            tsl = slice(t2 * 512, (t2 + 1) * 512)
            xh = xl_lo if t2 == 0 else xl_hi
            ps_x = psA.tile([P, TCW], f32, name="ps_x", tag="acc")
            for ci in range(CT):
                mm(ps_x, w1t_sb[:, ci, :], xh[:, ci, :], ci == 0, ci == CT - 1)
            nc.scalar.activation(out=xw1_sb[:, tsl], in_=ps_x, func=AF.Copy)

        chunks = [(h, t2 * 512, 512) for h in range(H - 1) for t2 in range(2)]
        # taper the final chunks: chain latency scales with width
        chunks += [(7, 0, 512), (7, 512, 256), (7, 768, 128), (7, 896, 128)]
        N = len(chunks)

        emit_xw1(0)
        emit_head_qk(0)
        emit_xw1(1)
        emit_head_qk(1)

        # vW1T = x.T @ (W1 Wv).T for all heads, fp8 DoubleRow: 2 channel
        # tiles contract per matmul ([P, 2, .] pair slices on both operands).
        # jh-major so the first w1wv half alone unblocks the first 8 groups.
        for jh in range(2):
            jsl = slice(jh * 512, (jh + 1) * 512)
            wv = w1wv0_sb if jh == 0 else w1wv1_sb
            for si in range(ST):
                ps_v = psA.tile([P, TCW], f32, name="ps_v", tag="acc")
                for a in range(CT // 2):
                    mm(
                        ps_v,
                        x8_sb[:, 2 * a : 2 * a + 2, si * P : (si + 1) * P],
                        wv[:, 2 * a : 2 * a + 2, :],
                        a == 0,
                        a == CT // 2 - 1,
                        perf_mode=DR,
                    )
                nc.scalar.activation(out=vw1t_sb[:, si, jsl], in_=ps_v, func=AF.Copy)

        # Steady-state iteration (S4 BEFORE S3: FC2(c-3)'s psum bank was
        # freed by recip(c-3) a full period ago, while Z(c-2) -- emitted
        # last -- keeps the in-order PE from stalling on the DVE chain).
        # Tail: S1 of the last two (128-wide) chunks is pulled one
        # iteration early (expp bufs=6 holds them) so their exp chains
        # complete before the drain's S2/S3/S4 cascade needs them.
        for i, c in enumerate(chunks):
            h, t_off, _ = c
            if t_off == 0:
                if h <= 5:
                    emit_head_dmas(h + 2)
                if 1 <= h <= 6:
                    emit_head_qk(h + 1)
            if i <= N - 3:
                emit_s1_half(c, True)
            if i >= 2:
                emit_s2_mm(chunks[i - 2], late=(i - 2 >= N - 5))
            if i <= N - 3:
                emit_s1_half(c, False)
            if i >= 3:
                emit_s4_s5(chunks[i - 3], late=(i - 3 >= N - 5))
            if i >= 2:
                emit_s2_s3(chunks[i - 2], late=(i - 2 >= N - 5))
            if i == N - 3:
                emit_s1_half(chunks[N - 2], True)
                emit_s1_half(chunks[N - 2], False)
            if i == N - 2:
                emit_s1_half(chunks[N - 1], True)
                emit_s1_half(chunks[N - 1], False)
        emit_s2_mm(chunks[N - 2], late=True)
        emit_s4_s5(chunks[N - 3], late=True)
        emit_s2_s3(chunks[N - 2], late=True)
        emit_s2_mm(chunks[N - 1], late=True)
        emit_s2_s3(chunks[N - 1], late=True)
        emit_s4_s5(chunks[N - 2], dve_add=True, dma_eng=nc.scalar, late=True)
        emit_s4_s5(chunks[N - 1], dve_add=True, dma_eng=nc.sync, late=True)

    nc.compile()
    return nc


def _prepare_inputs(inputs):
    import ml_dtypes

    f16 = np.float16
    bf16 = ml_dtypes.bfloat16
    f8 = ml_dtypes.float8_e4m3fn

    x = np.ascontiguousarray(np.asarray(inputs["x"], dtype=np.float32))
    Wq = np.asarray(inputs["Wq"], dtype=np.float32)
    bq = np.asarray(inputs["bq"], dtype=np.float32)
    Wk = np.asarray(inputs["Wk"], dtype=np.float32)
    bk = np.asarray(inputs["bk"], dtype=np.float32)
    Wv = np.asarray(inputs["Wv"], dtype=np.float32)
    bv = np.asarray(inputs["bv"], dtype=np.float32)
    gamma = np.asarray(inputs["gamma"], dtype=np.float32)
    W1 = np.asarray(inputs["W1"], dtype=np.float32)
    b1 = np.asarray(inputs["b1"], dtype=np.float32)
    W2 = np.asarray(inputs["W2"], dtype=np.float32)
    b2 = np.asarray(inputs["b2"], dtype=np.float32)

    # channel permutation c = 8p + ci: plain reshape of the (C, x) transposes
    # wqk[h, p, ci, 0:128] = Wq[h, :, 8p+ci]; 128:256 for Wk
    wqk = np.empty((H, P, CT, 256), dtype=np.float32)
    for h in range(H):
        wqk[h, :, :, 0:P] = Wq[h].T.reshape(P, CT, P)
        wqk[h, :, :, P : 2 * P] = Wk[h].T.reshape(P, CT, P)

    # w1wv[p, ci, h*128+j] = WVS * (W1 @ Wv[h]).T[8p+ci, j], fp8 with a x16
    # range pre-scale; compensated by shipping gamma/WVS
    w1wvT = np.concatenate([(W1 @ Wv[h]).T for h in range(H)], axis=1)  # (C, H*128)
    w1wv = w1wvT.reshape(P, CT, H * P) * WVS
    w1wv8 = w1wv.astype(f8)

    w1t = W1.T.reshape(P, CT, P)
    w2o = np.concatenate([W2.T, np.ones((P, P), dtype=np.float32)], axis=1)

    b1v = bv @ W1.T  # (H, P): b1v[h] = W1 @ bv[h]
    b1e = b1[None, :] + gamma[:, None] * b1v  # (H, P)
    # bias[:, h]=bq[h]; [:, H+h]=bk[h]; [:, 2H+h]=b1e[h]; [:, 3H+h]=gam/WVS; [:, 4H]=b2
    bias = np.empty((P, 4 * H + 1), dtype=np.float32)
    bias[:, 0:H] = bq.T
    bias[:, H : 2 * H] = bk.T
    bias[:, 2 * H : 3 * H] = b1e.T
    bias[:, 3 * H : 4 * H] = np.tile(gamma[None, :] / WVS, (P, 1))
    bias[:, 4 * H] = b2

    shared = {
        "wqk": np.ascontiguousarray(wqk.astype(f16)),
        "w1wv0": np.ascontiguousarray(w1wv8[:, :, 0:512]),
        "w1wv1": np.ascontiguousarray(w1wv8[:, :, 512:1024]),
        "w1t": np.ascontiguousarray(w1t.astype(f16)),
        "w2o": np.ascontiguousarray(w2o.astype(bf16)),
        "bias": bias,
    }
    in_maps = []
    for b in range(B):
        m = dict(shared)
        xr = x[b].reshape(P, CT, T)
        x16 = xr.astype(f16)
        m["x16lo"] = np.ascontiguousarray(x16[:, :, 0:512])
        m["x16hi"] = np.ascontiguousarray(x16[:, :, 512:1024])
        m["x8"] = np.ascontiguousarray(xr.astype(f8))
        in_maps.append(m)
    return in_maps


def kernel(**inputs):
    from concourse.bass_utils import run_bass_kernel_spmd

    if "nc" not in _module_cache:
        _module_cache["nc"] = _build_module()
    nc = _module_cache["nc"]

    in_maps = _prepare_inputs(inputs)
    res = run_bass_kernel_spmd(nc, in_maps, core_ids=list(range(B)))
    out = np.stack([res.results[b]["out"] for b in range(B)], axis=0)
    return out.astype(np.float32)

